# revision 35
# baseline (speedup 1.0000x reference)
"""MiniS4D Trainium2 kernel — channel-sharded SSM + AllToAll + batch-parallel mix.

Sharding: each of the 8 cores computes the (channel-independent) SSM /
depthwise-conv / GELU for its 64 channels across ALL 16 batches, with fat
512-column matmuls.  The GELU output y is resharded with two AllToAll
collectives (split by batch parity, the second overlapped with the mix) so
each core then holds all 512 channels for 2 batches, and runs the pointwise
channel-mix GEMM + GLU + mean + decode locally.

Math (per channel): conv with the TIME-REVERSED S4D kernel, decomposed into
chunks of T=128 (M=32 chunks):
  intra (lags 0..127): Toeplitz matmul with keff[0:128] (+D at lag 0);
    flipped operands: stationary = u-chunk [t, (4b x 32m)], moving = toep
    [t_in, t_out] so PSUM comes out [(b,m), t] — DMA-friendly, no transposes.
  inter (lags >= 128): rank-16 state expansion;
    Q[n,(b,m)] = sum_t r^t u[128m+t]   (B1 matmuls, 512 cols each)
    H[s] = prefix-sum over m<s of e^m Q[m], e = r^128   (DVE+Pool scan)
    G[s] = r^(L-1-128s) H[s]                            (postscale)
    y_inter[(b,m),t] = G-stationary x v1-moving matmul accumulated into the
    same PSUM tile as intra.

The mix GEMM runs in fp8-e4m3 DoubleRow mode (K=256 per pass) when FP8 is
set; y travels through the AllToAll in fp8 (half the wire bytes).  fp16
operands elsewhere, fp32 accumulation.  Output (16,1) assembled on host
from per-core (1,2); global batch b = 2*cid + bl.
"""
import sys
sys.path.insert(0, "/opt/trn_rl_repo")
import numpy as np

import concourse.bass as bass
import concourse.tile as tile
from concourse import bacc, mybir
from concourse import bass_utils

F32 = mybir.dt.float32
F16 = mybir.dt.float16
F8 = mybir.dt.float8e4
AF = mybir.ActivationFunctionType
ALU = mybir.AluOpType

FP8 = True              # fp8 y + fp8 DoubleRow mix

B, C, L, N = 16, 512, 4096, 8
T, M = 128, 32          # chunk length, number of chunks
S = 32                  # inter-state slots (slot s = chunk s), slot 0 == 0
NCORES = 8
CS = C // NCORES        # 64 channels per core
BL = 2                  # batches per core in the mix phase
NW = CS // 4            # 16 B1 waves (4 channels each: c = 4w + q)

# b' -> global batch permutation: even batches first (a2a chunk 0), then odd.
BPERM = np.r_[np.arange(0, B, 2), np.arange(1, B, 2)]

# swap re<->im rows within each 16-row half of every 32-partition group
SHUF32 = [(i // 16) * 16 + ((i % 16) + 8) % 16 for i in range(32)]

_compiled = None


def _prep(inputs):
    """Host-side parameter preparation (numpy, float64 internally)."""
    log_dt = inputs["log_dt"].astype(np.float64)
    A = -np.exp(inputs["log_A_real"].astype(np.float64)) \
        + 1j * inputs["A_imag"].astype(np.float64)            # (C, N)
    dt = np.exp(log_dt)
    r = np.exp(dt[:, None] * A)                                # (C, N)
    Bc = inputs["B_re"].astype(np.float64) + 1j * inputs["B_im"].astype(np.float64)
    Cc = inputs["C_re"].astype(np.float64) + 1j * inputs["C_im"].astype(np.float64)
    wv = Cc * (r - 1.0) / A * Bc                               # (C, N)
    rinv = 1.0 / r
    lags = np.arange(T)
    wL = wv * r ** (L - 1)
    keff = np.real(wL[:, :, None] * rinv[:, :, None] ** lags).sum(1)   # (C, T)
    keff[:, 0] += inputs["D"].astype(np.float64)

    toep = np.zeros((C, T, T), np.float16)
    for d in range(T):
        idx = np.arange(T - d)
        toep[:, idx, idx + d] = keff[:, d].astype(np.float16)[:, None]

    pw = r[:, :, None] ** lags                                 # (C, N, T)
    v2 = np.concatenate([pw.real, pw.imag], 1)                 # (C, 16, T)
    v2s = np.concatenate([pw.imag, pw.real], 1)                # row-swapped
    pw1 = wv[:, :, None] * rinv[:, :, None] ** lags
    v1 = np.concatenate([pw1.real, -pw1.imag], 1)              # (C, 16, T)

    e = r ** T                                                 # (C, N)
    Epow = e[:, :, None] ** np.arange(S - 1)                   # (C, N, 31): e^j
    Kp = r[:, :, None] ** (L - 1 - T * np.arange(S))           # (C, N, 32)
    Kp[:, :, 0] = 0.0

    # W is pre-scaled by WSC so fp8 quantization stays in the normal range;
    # 1/WSC is folded into the sigmoid scale and the decode weights.
    WSC = 1.0
    WT = np.ascontiguousarray(inputs["W_out"].T * WSC).astype(np.float16)
    # [ct2, p, i, o] with c = 256*ct2 + 128*i + p (DoubleRow k-pair layout)
    wmix8 = np.ascontiguousarray(
        WT.reshape(2, 2, 128, 1024).transpose(0, 2, 1, 3))
    b_out = inputs["b_out"].astype(np.float32)
    bouta = np.ascontiguousarray(b_out[:512].reshape(4, 128).T) * WSC   # (128, 4)
    boutg = np.ascontiguousarray(b_out[512:].reshape(4, 128).T)
    wd = (inputs["W_dec"][0].astype(np.float32) / (L * WSC)).reshape(4, 128).T
    wdec = np.ascontiguousarray(np.repeat(wd[:, None, :], BL, axis=1))  # (128, 2, 4)
    bdec = inputs["b_dec"].astype(np.float32).reshape(1, 1)

    u16 = inputs["u"].astype(np.float16)                                # (B, C, L)

    in_maps = []
    for cid in range(NCORES):
        c0 = CS * cid
        cs = slice(c0, c0 + CS)
        # uT[t, c, b', m] = u[BPERM[b'], c0+c, 128m+t]
        uT = np.ascontiguousarray(
            u16[BPERM, cs].reshape(B, CS, M, T).transpose(3, 1, 0, 2))
        toep2 = np.ascontiguousarray(toep[cs].transpose(1, 0, 2))       # [i, c, j]
        v2t = np.ascontiguousarray(
            v2[cs].transpose(2, 0, 1).astype(np.float16))               # [t, c, nh]
        v2ts = np.ascontiguousarray(
            v2s[cs].transpose(2, 0, 1).astype(np.float16))
        v1mt = np.zeros((128, NW, T), np.float16)
        e1h = np.zeros((128, NW, S - 1), np.float64)
        e2h = np.zeros((128, NW, S - 1), np.float64)
        k1h = np.zeros((128, NW, S), np.float64)
        k2h = np.zeros((128, NW, S), np.float64)
        for q in range(4):
            cq = c0 + 4 * np.arange(NW) + q                             # w -> channel
            v1mt[32 * q:32 * q + 16] = v1[cq].transpose(1, 0, 2)        # [16nh, w, t]
            Eq = Epow[cq].transpose(1, 0, 2)                            # (8n, w, 31)
            e1h[32 * q:32 * q + 8] = Eq.real
            e1h[32 * q + 8:32 * q + 16] = Eq.real
            e2h[32 * q:32 * q + 8] = -Eq.imag
            e2h[32 * q + 8:32 * q + 16] = Eq.imag
            Kq = Kp[cq].transpose(1, 0, 2)                              # (8n, w, 32)
            k1h[32 * q:32 * q + 8] = Kq.real
            k1h[32 * q + 8:32 * q + 16] = Kq.real
            k2h[32 * q:32 * q + 8] = -Kq.imag
            k2h[32 * q + 8:32 * q + 16] = Kq.imag
        in_maps.append(dict(
            uT=uT, toep2=toep2, v2t=v2t, v2ts=v2ts, v1mt=v1mt,
            e1h=e1h.astype(np.float16), e2h=e2h.astype(np.float16),
            k1h=k1h.astype(np.float16), k2h=k2h.astype(np.float16),
            wmix8=wmix8, bouta=bouta, boutg=boutg, wdec=wdec, bdec=bdec,
        ))
    return in_maps


def _build():
    nc = bacc.Bacc("TRN2", target_bir_lowering=False, debug=False,
                   num_devices=NCORES)
    d_uT = nc.dram_tensor("uT", [T, CS, B, M], F16, kind="ExternalInput").ap()
    d_toep = nc.dram_tensor("toep2", [T, CS, T], F16, kind="ExternalInput").ap()
    d_v2t = nc.dram_tensor("v2t", [T, CS, 16], F16, kind="ExternalInput").ap()
    d_v2ts = nc.dram_tensor("v2ts", [T, CS, 16], F16, kind="ExternalInput").ap()
    d_v1mt = nc.dram_tensor("v1mt", [128, NW, T], F16, kind="ExternalInput").ap()
    d_e1h = nc.dram_tensor("e1h", [128, NW, S - 1], F16, kind="ExternalInput").ap()
    d_e2h = nc.dram_tensor("e2h", [128, NW, S - 1], F16, kind="ExternalInput").ap()
    d_k1h = nc.dram_tensor("k1h", [128, NW, S], F16, kind="ExternalInput").ap()
    d_k2h = nc.dram_tensor("k2h", [128, NW, S], F16, kind="ExternalInput").ap()
    d_wmix8 = nc.dram_tensor("wmix8", [2, 128, 2, 1024], F16,
                             kind="ExternalInput").ap()
    d_bouta = nc.dram_tensor("bouta", [128, 4], F32, kind="ExternalInput").ap()
    d_boutg = nc.dram_tensor("boutg", [128, 4], F32, kind="ExternalInput").ap()
    d_wdec = nc.dram_tensor("wdec", [128, BL, 4], F32, kind="ExternalInput").ap()
    d_bdec = nc.dram_tensor("bdec", [1, 1], F32, kind="ExternalInput").ap()
    d_out = nc.dram_tensor("odec", [1, BL], F32, kind="ExternalOutput").ap()

    FY = F8 if FP8 else F16

    with tile.TileContext(nc) as tc:
        with tc.tile_pool(name="dram", bufs=1, space="DRAM") as dram, \
             tc.tile_pool(name="const", bufs=1) as constp:
            # a2a bounce buffers: par p holds batches of parity p
            # yin[bp, m, c, t] = y[c0+c, 2*bp + par, 128m + t]
            d_yin = [dram.tile([NCORES, M, CS, T], FY, tag=f"yin{p}",
                               name=f"yin{p}") for p in range(2)]
            d_yc = [dram.tile([NCORES, M, CS, T], FY, tag=f"yc{p}",
                              name=f"yc{p}") for p in range(2)]

            # small params first so B1 can start almost immediately
            bouta_sb = constp.tile([128, 4], F32)
            nc.scalar.dma_start(bouta_sb[:], d_bouta[:])
            boutg_sb = constp.tile([128, 4], F32)
            nc.scalar.dma_start(boutg_sb[:], d_boutg[:])
            wdec_sb = constp.tile([128, BL, 4], F32)
            nc.scalar.dma_start(wdec_sb[:], d_wdec[:])
            bdec_sb = constp.tile([1, 1], F32)
            nc.scalar.dma_start(bdec_sb[:], d_bdec[:])

            # ================= SSM phase (64 channels, 16 batches) ========
            with tc.tile_pool(name="uTp", bufs=1) as uTp, \
                 tc.tile_pool(name="prm", bufs=1) as prm, \
                 tc.tile_pool(name="Hp", bufs=1) as Hp:
                v2t = prm.tile([T, CS, 16], F16)
                nc.scalar.dma_start(v2t[:], d_v2t[:])
                v2ts = prm.tile([T, CS, 16], F16)
                nc.scalar.dma_start(v2ts[:], d_v2ts[:])
                scanmask = prm.tile([128, NW, B, S], F16)
                nc.vector.memset(scanmask[:], 1.0)
                nc.vector.memset(scanmask[:, :, :, 0:1], 0.0)
                e1h = prm.tile([128, NW, S - 1], F16)
                nc.scalar.dma_start(e1h[:], d_e1h[:])
                e2h = prm.tile([128, NW, S - 1], F16)
                nc.scalar.dma_start(e2h[:], d_e2h[:])
                k1h = prm.tile([128, NW, S], F16)
                nc.scalar.dma_start(k1h[:], d_k1h[:])
                k2h = prm.tile([128, NW, S], F16)
                nc.scalar.dma_start(k2h[:], d_k2h[:])
                v1mt = prm.tile([128, NW, T], F16)
                nc.scalar.dma_start(v1mt[:], d_v1mt[:])
                uT = uTp.tile([T, CS, B, M], F16)
                for ck in range(4):
                    sl = slice(16 * ck, 16 * ck + 16)
                    nc.sync.dma_start(uT[:, sl], d_uT[:, sl])
                toepT = prm.tile([T, CS, T], F16)
                for ck in range(4):
                    sl = slice(16 * ck, 16 * ck + 16)
                    nc.scalar.dma_start(toepT[:, sl], d_toep[:, sl])
                # mix weights: DMA now, but cast to fp8 only after the SSM
                # phase — a cast issued here would head-block the DVE queue
                # (and thus all of B1's prescale) on the 4 MiB weight DMA.
                wm = []
                wtmps = []
                for ct2 in range(2):
                    wtmp = constp.tile([128, 2, 1024], F16, name=f"wtmp{ct2}")
                    nc.scalar.dma_start(wtmp[:], d_wmix8[ct2])
                    wtmps.append(wtmp)
                    if FP8:
                        w8 = constp.tile([128, 2, 1024], F8, name=f"w8{ct2}")
                        wm.append(w8)
                    else:
                        wm.append(wtmp)

                H = Hp.tile([128, NW, B, S], F16)
                nc.vector.memset(H[:, :, :, 0:1], 0.0)

                # ---- B1: state matmuls + prescale (Q -> e^m Q) ----
                # The PE also emits a re<->im swapped copy of Q (second
                # stationary) so the complex prescale needs no DVE shuffle.
                with tc.tile_pool(name="hps", bufs=3, space="PSUM") as hps, \
                     tc.tile_pool(name="b1s", bufs=3) as b1s, \
                     tc.tile_pool(name="pss", bufs=2) as pss:
                    for half in range(2):
                        for w in range(8 * half, 8 * half + 8):
                            hb = hps.tile([128, B, M], F32, tag="hb")
                            hbs = hps.tile([128, B, M], F32, tag="hbs")
                            for q in range(4):
                                c = 4 * w + q
                                nc.tensor.matmul(
                                    hb[32 * q:32 * q + 16, :, :],
                                    v2t[:, c, :], uT[:, c, :, :],
                                    start=True, stop=True,
                                    tile_position=(0, 32 * q))
                                nc.tensor.matmul(
                                    hbs[32 * q:32 * q + 16, :, :],
                                    v2ts[:, c, :], uT[:, c, :, :],
                                    start=True, stop=True,
                                    tile_position=(0, 32 * q))
                            t1 = b1s.tile([128, B, S - 1], F16, tag="t1")
                            e1b = e1h[:, w].unsqueeze(1) \
                                .broadcast_to([128, B, S - 1])
                            e2b = e2h[:, w].unsqueeze(1) \
                                .broadcast_to([128, B, S - 1])
                            nc.vector.tensor_mul(t1[:], hb[:, :, 0:S - 1], e1b)
                            nc.vector.tensor_mul(hbs[:, :, 0:S - 1],
                                                 hbs[:, :, 0:S - 1], e2b)
                            nc.vector.tensor_add(H[:, w, :, 1:S], t1[:],
                                                 hbs[:, :, 0:S - 1])
                        # fused prefix-scan for this half (mask resets the
                        # carry at each (w, b) group boundary), then the
                        # postscale — so B2 can start on the first half while
                        # the second half's prescale is still running.
                        hsl = slice(8 * half, 8 * half + 8)
                        nc.vector.tensor_tensor_scan(
                            H[:, hsl].rearrange("p a b c -> p (a b c)"),
                            scanmask[:, hsl].rearrange("p a b c -> p (a b c)"),
                            H[:, hsl].rearrange("p a b c -> p (a b c)"),
                            0.0, op0=ALU.mult, op1=ALU.add)
                        for wg in (2 * half, 2 * half + 1):
                            wsl = slice(4 * wg, 4 * wg + 4)
                            sw2 = pss.tile([128, 4, B, S], F16, tag="sw2")
                            t2 = pss.tile([128, 4, B, S], F16, tag="t2")
                            hgc = H[:, wsl]
                            k1b = k1h[:, wsl].unsqueeze(2) \
                                .broadcast_to([128, 4, B, S])
                            k2b = k2h[:, wsl].unsqueeze(2) \
                                .broadcast_to([128, 4, B, S])
                            nc.vector.stream_shuffle(sw2[:], hgc, SHUF32)
                            nc.vector.tensor_mul(t2[:], hgc, k1b)
                            nc.gpsimd.tensor_mul(sw2[:], sw2[:], k2b)
                            nc.vector.tensor_add(hgc, t2[:], sw2[:])

                # ---- B2: intra Toeplitz + inter expand + GELU -> y_in ----
                # PSUM out [(4b' x 32m), t] per (channel, batch-group);
                # intra x4 then inter x4 so LDWEIGHTS pipelines with moving.
                with tc.tile_pool(name="yps", bufs=6, space="PSUM") as ypsp, \
                     tc.tile_pool(name="stg", bufs=4) as stgp:
                    # cw-outer so B2 starts right after the first half's
                    # postscale, overlapping the second half's B1/prescale.
                    for cw in range(NW):
                        for g in range(4):      # b' 4g..4g+4; parity par=g//2
                            par = g // 2
                            bsl = slice(4 * g, 4 * g + 4)
                            yps = ypsp.tile([128, 4, T], F32)
                            for q in range(4):
                                c = 4 * cw + q
                                nc.tensor.matmul(
                                    yps[:, q, :],
                                    uT[:, c, bsl, :], toepT[:, c, :],
                                    start=True, stop=False)
                                nc.tensor.matmul(
                                    yps[:, q, :],
                                    H[32 * q:32 * q + 16, cw, bsl, 0:S],
                                    v1mt[32 * q:32 * q + 16, cw, :],
                                    start=False, stop=True,
                                    tile_position=(32 * q, 0))
                            st = stgp.tile([128, 4, T], FY)
                            nc.scalar.activation(st[:], yps[:], AF.Gelu)
                            # dst [bp(4), m, c(4), t] <- src [(4b',32m),(4c,t)]
                            bp0 = 4 * (g % 2)
                            nc.sync.dma_start(
                                d_yin[par][bp0:bp0 + 4, :, 4 * cw:4 * cw + 4],
                                st[:])
                            if cw == NW - 1 and (g == 1 or g == 3):
                                nc.gpsimd.collective_compute(
                                    "AllToAll",
                                    mybir.AluOpType.bypass,
                                    replica_groups=[list(range(NCORES))],
                                    ins=[d_yin[par][:].opt()],
                                    outs=[d_yc[par][:].opt()],
                                )

            # ================= Mix phase (2 batches, 512 channels) ========
            if FP8:
                for ct2 in range(2):
                    nc.vector.tensor_copy(wm[ct2][:], wtmps[ct2][:])
            with tc.tile_pool(name="ytp", bufs=1) as ytp, \
                 tc.tile_pool(name="sgp", bufs=4) as sgp, \
                 tc.tile_pool(name="m1p", bufs=1) as m1p:
                M1 = m1p.tile([128, BL, 4, 8], F32)
                # prefetch y tiles on the gpsimd queue: it is empty after the
                # (non-blocking) a2a triggers, so the loads dispatch the
                # moment each collective's completion semaphore fires, and
                # their waits cannot head-block GELUs or sigmoids.
                ytF2 = {}
                for bl in range(BL):
                    eng = nc.gpsimd
                    for ct2 in range(2):
                        t = ytp.tile([128, 2, M, T], FY, name=f"yt{bl}{ct2}")
                        for i in range(2):
                            for h in range(2):
                                eng.dma_start(
                                    t[64 * h:64 * h + 64, i],
                                    d_yc[bl][4 * ct2 + 2 * i + h]
                                    .transpose([1, 0, 2]))
                        ytF2[(bl, ct2)] = t
                with tc.tile_pool(name="zps", bufs=2, space="PSUM") as zpsp:
                    for bl in range(BL):
                        for pr in range(4):
                            for pp in range(4):
                                lcs = (2 * pp, 2 * pp + 1)
                                za2 = zpsp.tile([128, 2, 512], F32, tag="za")
                                zg2 = zpsp.tile([128, 2, 512], F32, tag="zg")
                                for side, zt in ((0, za2), (1, zg2)):
                                    ot = pr + 4 * side
                                    osl = slice(128 * ot, 128 * ot + 128)
                                    for ih, lc in enumerate(lcs):
                                        csl = slice(4 * lc, 4 * lc + 4)
                                        if FP8:
                                            for ct2 in range(2):
                                                nc.tensor.matmul(
                                                    zt[:, ih],
                                                    wm[ct2][:, :, osl],
                                                    ytF2[(bl, ct2)][:, :, csl, :]
                                                    .rearrange("p i a b -> p i (a b)"),
                                                    start=(ct2 == 0),
                                                    stop=(ct2 == 1),
                                                    perf_mode=mybir.MatmulPerfMode.DoubleRow)
                                        else:
                                            for ct2 in range(2):
                                                for i in range(2):
                                                    nc.tensor.matmul(
                                                        zt[:, ih],
                                                        wm[ct2][:, i, osl],
                                                        ytF2[(bl, ct2)][:, i, csl, :]
                                                        .rearrange("p a b -> p (a b)"),
                                                        start=(ct2 == 0 and i == 0),
                                                        stop=(ct2 == 1 and i == 1))
                                sg2 = sgp.tile([128, 2, 512], F16, tag="sg")
                                nc.scalar.activation(
                                    sg2[:], zg2[:], AF.Sigmoid,
                                    bias=boutg_sb[:, pr:pr + 1],
                                    scale=1.0)
                                for ih, lc in enumerate(lcs):
                                    scr = sgp.tile([128, 512], F16, tag="scr")
                                    nc.vector.scalar_tensor_tensor(
                                        scr[:], za2[:, ih],
                                        bouta_sb[:, pr:pr + 1],
                                        sg2[:, ih],
                                        op0=ALU.add, op1=ALU.mult,
                                        accum_out=M1[:, bl:bl + 1, pr:pr + 1,
                                                     lc:lc + 1].squeeze()
                                        .unsqueeze(1))

                # ---- decode ----
                with tc.tile_pool(name="dps", bufs=1, space="PSUM") as dpsp:
                    R1 = m1p.tile([128, BL, 4], F32)
                    nc.vector.reduce_sum(R1[:], M1[:], axis=mybir.AxisListType.X)
                    R2 = m1p.tile([128, BL, 4], F32)
                    nc.vector.tensor_mul(R2[:], R1[:], wdec_sb[:])
                    R3 = m1p.tile([128, BL], F32)
                    nc.vector.reduce_sum(R3[:], R2[:], axis=mybir.AxisListType.X)
                    ones = m1p.tile([128, 1], F32)
                    nc.vector.memset(ones[:], 1.0)
                    dp = dpsp.tile([1, BL], F32)
                    nc.tensor.matmul(dp[:], ones[:], R3[:], start=True, stop=True)
                    osb = m1p.tile([1, BL], F32)
                    nc.vector.tensor_scalar_add(osb[:], dp[:], bdec_sb[:, 0:1])
                    nc.sync.dma_start(d_out[:], osb[:])

    nc.compile()
    return nc


def _get_compiled():
    global _compiled
    if _compiled is None:
        _compiled = _build()
    return _compiled


def _run(inputs, trace=False, **kw):
    in_maps = _prep(inputs)
    nc = _get_compiled()
    return bass_utils.run_bass_kernel_spmd(
        nc, in_maps, core_ids=list(range(NCORES)), trace=trace, **kw)


def kernel(**inputs):
    inputs = {k: np.asarray(v) for k, v in inputs.items()}
    res = _run(inputs)
    out = np.empty((B, 1), np.float32)
    for cid in range(NCORES):
        out[2 * cid, 0] = res.results[cid]["odec"][0, 0]
        out[2 * cid + 1, 0] = res.results[cid]["odec"][0, 1]
    return out


# revision 38
# speedup vs baseline: 1.0875x; 1.0875x over previous
"""MiniS4D Trainium2 kernel — channel-sharded SSM + AllToAll + batch-parallel mix.

Sharding: each of the 8 cores computes the (channel-independent) SSM /
depthwise-conv / GELU for its 64 channels across ALL 16 batches, with fat
512-column matmuls.  The GELU output y is resharded with two AllToAll
collectives (split by batch parity, the second overlapped with the mix) so
each core then holds all 512 channels for 2 batches, and runs the pointwise
channel-mix GEMM + GLU + mean + decode locally.

Math (per channel): conv with the TIME-REVERSED S4D kernel, decomposed into
chunks of T=128 (M=32 chunks):
  intra (lags 0..127): Toeplitz matmul with keff[0:128] (+D at lag 0);
    flipped operands: stationary = u-chunk [t, (4b x 32m)], moving = toep
    [t_in, t_out] so PSUM comes out [(b,m), t] — DMA-friendly, no transposes.
  inter (lags >= 128): rank-16 state expansion;
    Q[n,(b,m)] = sum_t r^t u[128m+t]   (B1 matmuls, 512 cols each)
    H[s] = prefix-sum over m<s of e^m Q[m], e = r^128   (DVE+Pool scan)
    G[s] = r^(L-1-128s) H[s]                            (postscale)
    y_inter[(b,m),t] = G-stationary x v1-moving matmul accumulated into the
    same PSUM tile as intra.

The mix GEMM runs in fp8-e4m3 DoubleRow mode (K=256 per pass) when FP8 is
set; y travels through the AllToAll in fp8 (half the wire bytes).  fp16
operands elsewhere, fp32 accumulation.  Output (16,1) assembled on host
from per-core (1,2); global batch b = 2*cid + bl.
"""
import sys
sys.path.insert(0, "/opt/trn_rl_repo")
import numpy as np

import concourse.bass as bass
import concourse.tile as tile
from concourse import bacc, mybir
from concourse import bass_utils

F32 = mybir.dt.float32
F16 = mybir.dt.float16
F8 = mybir.dt.float8e4
AF = mybir.ActivationFunctionType
ALU = mybir.AluOpType

FP8 = True              # fp8 y + fp8 DoubleRow mix

B, C, L, N = 16, 512, 4096, 8
T, M = 128, 32          # chunk length, number of chunks
S = 32                  # inter-state slots (slot s = chunk s), slot 0 == 0
NCORES = 8
CS = C // NCORES        # 64 channels per core
BL = 2                  # batches per core in the mix phase
NW = CS // 4            # 16 B1 waves (4 channels each: c = 4w + q)

# b' -> global batch permutation: even batches first (a2a chunk 0), then odd.
BPERM = np.r_[np.arange(0, B, 2), np.arange(1, B, 2)]

# swap re<->im rows within each 16-row half of every 32-partition group
SHUF32 = [(i // 16) * 16 + ((i % 16) + 8) % 16 for i in range(32)]

_compiled = None


def _prep(inputs):
    """Host-side parameter preparation (numpy, float64 internally)."""
    log_dt = inputs["log_dt"].astype(np.float64)
    A = -np.exp(inputs["log_A_real"].astype(np.float64)) \
        + 1j * inputs["A_imag"].astype(np.float64)            # (C, N)
    dt = np.exp(log_dt)
    r = np.exp(dt[:, None] * A)                                # (C, N)
    Bc = inputs["B_re"].astype(np.float64) + 1j * inputs["B_im"].astype(np.float64)
    Cc = inputs["C_re"].astype(np.float64) + 1j * inputs["C_im"].astype(np.float64)
    wv = Cc * (r - 1.0) / A * Bc                               # (C, N)
    rinv = 1.0 / r
    lags = np.arange(T)
    wL = wv * r ** (L - 1)
    keff = np.real(wL[:, :, None] * rinv[:, :, None] ** lags).sum(1)   # (C, T)
    keff[:, 0] += inputs["D"].astype(np.float64)

    toep = np.zeros((C, T, T), np.float16)
    for d in range(T):
        idx = np.arange(T - d)
        toep[:, idx, idx + d] = keff[:, d].astype(np.float16)[:, None]

    pw = r[:, :, None] ** lags                                 # (C, N, T)
    v2 = np.concatenate([pw.real, pw.imag], 1)                 # (C, 16, T)
    v2s = np.concatenate([pw.imag, pw.real], 1)                # row-swapped
    pw1 = wv[:, :, None] * rinv[:, :, None] ** lags
    v1 = np.concatenate([pw1.real, -pw1.imag], 1)              # (C, 16, T)

    e = r ** T                                                 # (C, N)
    Epow = e[:, :, None] ** np.arange(S - 1)                   # (C, N, 31): e^j
    Kp = r[:, :, None] ** (L - 1 - T * np.arange(S))           # (C, N, 32)
    Kp[:, :, 0] = 0.0

    # W is pre-scaled by WSC so fp8 quantization stays in the normal range;
    # 1/WSC is folded into the sigmoid scale and the decode weights.
    WSC = 1.0
    WT = np.ascontiguousarray(inputs["W_out"].T * WSC).astype(np.float16)
    # [ct2, p, i, o] with c = 256*ct2 + 128*i + p (DoubleRow k-pair layout)
    wmix8 = np.ascontiguousarray(
        WT.reshape(2, 2, 128, 1024).transpose(0, 2, 1, 3))
    b_out = inputs["b_out"].astype(np.float32)
    bouta = np.ascontiguousarray(b_out[:512].reshape(4, 128).T) * WSC   # (128, 4)
    boutg = np.ascontiguousarray(b_out[512:].reshape(4, 128).T)
    wd = (inputs["W_dec"][0].astype(np.float32) / (L * WSC)).reshape(4, 128).T
    wdec = np.ascontiguousarray(np.repeat(wd[:, None, :], BL, axis=1))  # (128, 2, 4)
    bdec = inputs["b_dec"].astype(np.float32).reshape(1, 1)

    u16 = inputs["u"].astype(np.float16)                                # (B, C, L)

    in_maps = []
    for cid in range(NCORES):
        c0 = CS * cid
        cs = slice(c0, c0 + CS)
        # uT[t, c, b', m] = u[BPERM[b'], c0+c, 128m+t]
        uT = np.ascontiguousarray(
            u16[BPERM, cs].reshape(B, CS, M, T).transpose(3, 1, 0, 2))
        toep2 = np.ascontiguousarray(toep[cs].transpose(1, 0, 2))       # [i, c, j]
        v2t = np.ascontiguousarray(
            v2[cs].transpose(2, 0, 1).astype(np.float16))               # [t, c, nh]
        v2ts = np.ascontiguousarray(
            v2s[cs].transpose(2, 0, 1).astype(np.float16))
        v1mt = np.zeros((128, NW, T), np.float16)
        e1h = np.zeros((128, NW, S - 1), np.float64)
        e2h = np.zeros((128, NW, S - 1), np.float64)
        k1h = np.zeros((128, NW, S), np.float64)
        k2h = np.zeros((128, NW, S), np.float64)
        for q in range(4):
            cq = c0 + 4 * np.arange(NW) + q                             # w -> channel
            v1mt[32 * q:32 * q + 16] = v1[cq].transpose(1, 0, 2)        # [16nh, w, t]
            Eq = Epow[cq].transpose(1, 0, 2)                            # (8n, w, 31)
            e1h[32 * q:32 * q + 8] = Eq.real
            e1h[32 * q + 8:32 * q + 16] = Eq.real
            e2h[32 * q:32 * q + 8] = -Eq.imag
            e2h[32 * q + 8:32 * q + 16] = Eq.imag
            Kq = Kp[cq].transpose(1, 0, 2)                              # (8n, w, 32)
            k1h[32 * q:32 * q + 8] = Kq.real
            k1h[32 * q + 8:32 * q + 16] = Kq.real
            k2h[32 * q:32 * q + 8] = -Kq.imag
            k2h[32 * q + 8:32 * q + 16] = Kq.imag
        in_maps.append(dict(
            uT=uT, toep2=toep2, v2t=v2t, v2ts=v2ts, v1mt=v1mt,
            e1h=e1h.astype(np.float16), e2h=e2h.astype(np.float16),
            k1h=k1h.astype(np.float16), k2h=k2h.astype(np.float16),
            wmix8=wmix8, bouta=bouta, boutg=boutg, wdec=wdec, bdec=bdec,
        ))
    return in_maps


def _build():
    nc = bacc.Bacc("TRN2", target_bir_lowering=False, debug=False,
                   num_devices=NCORES)
    d_uT = nc.dram_tensor("uT", [T, CS, B, M], F16, kind="ExternalInput").ap()
    d_toep = nc.dram_tensor("toep2", [T, CS, T], F16, kind="ExternalInput").ap()
    d_v2t = nc.dram_tensor("v2t", [T, CS, 16], F16, kind="ExternalInput").ap()
    d_v2ts = nc.dram_tensor("v2ts", [T, CS, 16], F16, kind="ExternalInput").ap()
    d_v1mt = nc.dram_tensor("v1mt", [128, NW, T], F16, kind="ExternalInput").ap()
    d_e1h = nc.dram_tensor("e1h", [128, NW, S - 1], F16, kind="ExternalInput").ap()
    d_e2h = nc.dram_tensor("e2h", [128, NW, S - 1], F16, kind="ExternalInput").ap()
    d_k1h = nc.dram_tensor("k1h", [128, NW, S], F16, kind="ExternalInput").ap()
    d_k2h = nc.dram_tensor("k2h", [128, NW, S], F16, kind="ExternalInput").ap()
    d_wmix8 = nc.dram_tensor("wmix8", [2, 128, 2, 1024], F16,
                             kind="ExternalInput").ap()
    d_bouta = nc.dram_tensor("bouta", [128, 4], F32, kind="ExternalInput").ap()
    d_boutg = nc.dram_tensor("boutg", [128, 4], F32, kind="ExternalInput").ap()
    d_wdec = nc.dram_tensor("wdec", [128, BL, 4], F32, kind="ExternalInput").ap()
    d_bdec = nc.dram_tensor("bdec", [1, 1], F32, kind="ExternalInput").ap()
    d_out = nc.dram_tensor("odec", [1, BL], F32, kind="ExternalOutput").ap()

    FY = F8 if FP8 else F16

    with tile.TileContext(nc) as tc:
        with tc.tile_pool(name="dram", bufs=1, space="DRAM") as dram, \
             tc.tile_pool(name="const", bufs=1) as constp:
            # a2a bounce buffers: par p holds batches of parity p
            # yin[bp, m, c, t] = y[c0+c, 2*bp + par, 128m + t]
            d_yin = [dram.tile([NCORES, M, CS, T], FY, tag=f"yin{p}",
                               name=f"yin{p}") for p in range(2)]
            d_yc = [dram.tile([NCORES, M, CS, T], FY, tag=f"yc{p}",
                              name=f"yc{p}") for p in range(2)]

            # small params first so B1 can start almost immediately
            bouta_sb = constp.tile([128, 4], F32)
            nc.scalar.dma_start(bouta_sb[:], d_bouta[:])
            boutg_sb = constp.tile([128, 4], F32)
            nc.scalar.dma_start(boutg_sb[:], d_boutg[:])
            wdec_sb = constp.tile([128, BL, 4], F32)
            nc.scalar.dma_start(wdec_sb[:], d_wdec[:])
            bdec_sb = constp.tile([1, 1], F32)
            nc.scalar.dma_start(bdec_sb[:], d_bdec[:])

            # ================= SSM phase (64 channels, 16 batches) ========
            with tc.tile_pool(name="uTp", bufs=1) as uTp, \
                 tc.tile_pool(name="prm", bufs=1) as prm, \
                 tc.tile_pool(name="Hp", bufs=1) as Hp:
                v2t = prm.tile([T, CS, 16], F16)
                nc.scalar.dma_start(v2t[:], d_v2t[:])
                v2ts = prm.tile([T, CS, 16], F16)
                nc.scalar.dma_start(v2ts[:], d_v2ts[:])
                scanmask = prm.tile([128, NW, B, S], F16)
                nc.vector.memset(scanmask[:], 1.0)
                nc.vector.memset(scanmask[:, :, :, 0:1], 0.0)
                e1h = prm.tile([128, NW, S - 1], F16)
                nc.scalar.dma_start(e1h[:], d_e1h[:])
                e2h = prm.tile([128, NW, S - 1], F16)
                nc.scalar.dma_start(e2h[:], d_e2h[:])
                k1h = prm.tile([128, NW, S], F16)
                nc.scalar.dma_start(k1h[:], d_k1h[:])
                k2h = prm.tile([128, NW, S], F16)
                nc.scalar.dma_start(k2h[:], d_k2h[:])
                v1mt = prm.tile([128, NW, T], F16)
                nc.scalar.dma_start(v1mt[:], d_v1mt[:])
                uT = uTp.tile([T, CS, B, M], F16)
                for ck in range(4):
                    sl = slice(16 * ck, 16 * ck + 16)
                    nc.sync.dma_start(uT[:, sl], d_uT[:, sl])
                toepT = prm.tile([T, CS, T], F16)
                for ck in range(4):
                    sl = slice(16 * ck, 16 * ck + 16)
                    nc.scalar.dma_start(toepT[:, sl], d_toep[:, sl])
                # mix weights: DMA now, but cast to fp8 only after the SSM
                # phase — a cast issued here would head-block the DVE queue
                # (and thus all of B1's prescale) on the 4 MiB weight DMA.
                wm = []
                wtmps = []
                for ct2 in range(2):
                    wtmp = constp.tile([128, 2, 1024], F16, name=f"wtmp{ct2}")
                    nc.scalar.dma_start(wtmp[:], d_wmix8[ct2])
                    wtmps.append(wtmp)
                    if FP8:
                        w8 = constp.tile([128, 2, 1024], F8, name=f"w8{ct2}")
                        wm.append(w8)
                    else:
                        wm.append(wtmp)

                H = Hp.tile([128, NW, B, S], F16)
                nc.vector.memset(H[:, :, :, 0:1], 0.0)

                # ---- B1: state matmuls + prescale (Q -> e^m Q) ----
                # The PE also emits a re<->im swapped copy of Q (second
                # stationary) so the complex prescale needs no DVE shuffle.
                with tc.tile_pool(name="hps", bufs=3, space="PSUM") as hps, \
                     tc.tile_pool(name="b1s", bufs=3) as b1s, \
                     tc.tile_pool(name="pss", bufs=2) as pss:
                    for half in range(2):
                        for w in range(8 * half, 8 * half + 8):
                            hb = hps.tile([128, B, M], F32, tag="hb")
                            hbs = hps.tile([128, B, M], F32, tag="hbs")
                            for q in range(4):
                                c = 4 * w + q
                                nc.tensor.matmul(
                                    hb[32 * q:32 * q + 16, :, :],
                                    v2t[:, c, :], uT[:, c, :, :],
                                    start=True, stop=True,
                                    tile_position=(0, 32 * q))
                                nc.tensor.matmul(
                                    hbs[32 * q:32 * q + 16, :, :],
                                    v2ts[:, c, :], uT[:, c, :, :],
                                    start=True, stop=True,
                                    tile_position=(0, 32 * q))
                            t1 = b1s.tile([128, B, S - 1], F16, tag="t1")
                            e1b = e1h[:, w].unsqueeze(1) \
                                .broadcast_to([128, B, S - 1])
                            e2b = e2h[:, w].unsqueeze(1) \
                                .broadcast_to([128, B, S - 1])
                            nc.vector.tensor_mul(t1[:], hb[:, :, 0:S - 1], e1b)
                            nc.vector.tensor_mul(hbs[:, :, 0:S - 1],
                                                 hbs[:, :, 0:S - 1], e2b)
                            nc.vector.tensor_add(H[:, w, :, 1:S], t1[:],
                                                 hbs[:, :, 0:S - 1])
                        # fused prefix-scan + postscale for this half so B2
                        # par-0 tiles can start while half 1 is in flight
                        hsl = slice(8 * half, 8 * half + 8)
                        nc.vector.tensor_tensor_scan(
                            H[:, hsl].rearrange("p a b c -> p (a b c)"),
                            scanmask[:, hsl].rearrange("p a b c -> p (a b c)"),
                            H[:, hsl].rearrange("p a b c -> p (a b c)"),
                            0.0, op0=ALU.mult, op1=ALU.add)
                        for wg in (2 * half, 2 * half + 1):
                            wsl = slice(4 * wg, 4 * wg + 4)
                            sw2 = pss.tile([128, 4, B, S], F16, tag="sw2")
                            t2 = pss.tile([128, 4, B, S], F16, tag="t2")
                            hgc = H[:, wsl]
                            k1b = k1h[:, wsl].unsqueeze(2) \
                                .broadcast_to([128, 4, B, S])
                            k2b = k2h[:, wsl].unsqueeze(2) \
                                .broadcast_to([128, 4, B, S])
                            nc.vector.stream_shuffle(sw2[:], hgc, SHUF32)
                            nc.vector.tensor_mul(t2[:], hgc, k1b)
                            nc.gpsimd.tensor_mul(sw2[:], sw2[:], k2b)
                            nc.vector.tensor_add(hgc, t2[:], sw2[:])

                # ---- B2: intra Toeplitz + inter expand + GELU -> y_in ----
                # PSUM out [(4b' x 32m), t] per (channel, batch-group);
                # intra x4 then inter x4 so LDWEIGHTS pipelines with moving.
                with tc.tile_pool(name="yps", bufs=6, space="PSUM") as ypsp, \
                     tc.tile_pool(name="stg", bufs=4) as stgp:
                    # parity-outer: finish all par-0 tiles first so a2a #0
                    # fires as early as possible; cw order inside lets the
                    # first channel-groups overlap B1's second half.
                    for par in range(2):
                        for cw in range(NW):
                            for g in (2 * par, 2 * par + 1):
                                bsl = slice(4 * g, 4 * g + 4)
                                yps = ypsp.tile([128, 4, T], F32)
                                for q in range(4):
                                    c = 4 * cw + q
                                    nc.tensor.matmul(
                                        yps[:, q, :],
                                        uT[:, c, bsl, :], toepT[:, c, :],
                                        start=True, stop=False)
                                    nc.tensor.matmul(
                                        yps[:, q, :],
                                        H[32 * q:32 * q + 16, cw, bsl, 0:S],
                                        v1mt[32 * q:32 * q + 16, cw, :],
                                        start=False, stop=True,
                                        tile_position=(32 * q, 0))
                                st = stgp.tile([128, 4, T], FY)
                                nc.scalar.activation(st[:], yps[:], AF.Gelu)
                                # [bp(4), m, c(4), t] <- [(4b',32m),(4c,t)]
                                bp0 = 4 * (g % 2)
                                nc.sync.dma_start(
                                    d_yin[par][bp0:bp0 + 4, :,
                                               4 * cw:4 * cw + 4],
                                    st[:])
                        nc.gpsimd.collective_compute(
                            "AllToAll",
                            mybir.AluOpType.bypass,
                            replica_groups=[list(range(NCORES))],
                            ins=[d_yin[par][:].opt()],
                            outs=[d_yc[par][:].opt()],
                        )

            # ================= Mix phase (2 batches, 512 channels) ========
            if FP8:
                for ct2 in range(2):
                    nc.vector.tensor_copy(wm[ct2][:], wtmps[ct2][:])
            with tc.tile_pool(name="ytp", bufs=1) as ytp, \
                 tc.tile_pool(name="sgp", bufs=4) as sgp, \
                 tc.tile_pool(name="m1p", bufs=1) as m1p:
                M1 = m1p.tile([128, BL, 4, 8], F32)
                # prefetch y tiles on the gpsimd queue: it is empty after the
                # (non-blocking) a2a triggers, so the loads dispatch the
                # moment each collective's completion semaphore fires, and
                # their waits cannot head-block GELUs or sigmoids.
                ytF2 = {}
                for bl in range(BL):
                    eng = nc.gpsimd
                    for ct2 in range(2):
                        t = ytp.tile([128, 2, M, T], FY, name=f"yt{bl}{ct2}")
                        for i in range(2):
                            for h in range(2):
                                eng.dma_start(
                                    t[64 * h:64 * h + 64, i],
                                    d_yc[bl][4 * ct2 + 2 * i + h]
                                    .transpose([1, 0, 2]))
                        ytF2[(bl, ct2)] = t
                with tc.tile_pool(name="zps", bufs=2, space="PSUM") as zpsp:
                    for bl in range(BL):
                        for pr in range(4):
                            for pp in range(4):
                                lcs = (2 * pp, 2 * pp + 1)
                                za2 = zpsp.tile([128, 2, 512], F32, tag="za")
                                zg2 = zpsp.tile([128, 2, 512], F32, tag="zg")
                                for side, zt in ((0, za2), (1, zg2)):
                                    ot = pr + 4 * side
                                    osl = slice(128 * ot, 128 * ot + 128)
                                    for ih, lc in enumerate(lcs):
                                        csl = slice(4 * lc, 4 * lc + 4)
                                        if FP8:
                                            for ct2 in range(2):
                                                nc.tensor.matmul(
                                                    zt[:, ih],
                                                    wm[ct2][:, :, osl],
                                                    ytF2[(bl, ct2)][:, :, csl, :]
                                                    .rearrange("p i a b -> p i (a b)"),
                                                    start=(ct2 == 0),
                                                    stop=(ct2 == 1),
                                                    perf_mode=mybir.MatmulPerfMode.DoubleRow)
                                        else:
                                            for ct2 in range(2):
                                                for i in range(2):
                                                    nc.tensor.matmul(
                                                        zt[:, ih],
                                                        wm[ct2][:, i, osl],
                                                        ytF2[(bl, ct2)][:, i, csl, :]
                                                        .rearrange("p a b -> p (a b)"),
                                                        start=(ct2 == 0 and i == 0),
                                                        stop=(ct2 == 1 and i == 1))
                                sg2 = sgp.tile([128, 2, 512], F16, tag="sg")
                                nc.scalar.activation(
                                    sg2[:], zg2[:], AF.Sigmoid,
                                    bias=boutg_sb[:, pr:pr + 1],
                                    scale=1.0)
                                for ih, lc in enumerate(lcs):
                                    scr = sgp.tile([128, 512], F16, tag="scr")
                                    nc.vector.scalar_tensor_tensor(
                                        scr[:], za2[:, ih],
                                        bouta_sb[:, pr:pr + 1],
                                        sg2[:, ih],
                                        op0=ALU.add, op1=ALU.mult,
                                        accum_out=M1[:, bl:bl + 1, pr:pr + 1,
                                                     lc:lc + 1].squeeze()
                                        .unsqueeze(1))

                # ---- decode ----
                with tc.tile_pool(name="dps", bufs=1, space="PSUM") as dpsp:
                    R1 = m1p.tile([128, BL, 4], F32)
                    nc.vector.reduce_sum(R1[:], M1[:], axis=mybir.AxisListType.X)
                    R2 = m1p.tile([128, BL, 4], F32)
                    nc.vector.tensor_mul(R2[:], R1[:], wdec_sb[:])
                    R3 = m1p.tile([128, BL], F32)
                    nc.vector.reduce_sum(R3[:], R2[:], axis=mybir.AxisListType.X)
                    ones = m1p.tile([128, 1], F32)
                    nc.vector.memset(ones[:], 1.0)
                    dp = dpsp.tile([1, BL], F32)
                    nc.tensor.matmul(dp[:], ones[:], R3[:], start=True, stop=True)
                    osb = m1p.tile([1, BL], F32)
                    nc.vector.tensor_scalar_add(osb[:], dp[:], bdec_sb[:, 0:1])
                    nc.sync.dma_start(d_out[:], osb[:])

    nc.compile()
    return nc


def _get_compiled():
    global _compiled
    if _compiled is None:
        _compiled = _build()
    return _compiled


def _run(inputs, trace=False, **kw):
    in_maps = _prep(inputs)
    nc = _get_compiled()
    return bass_utils.run_bass_kernel_spmd(
        nc, in_maps, core_ids=list(range(NCORES)), trace=trace, **kw)


def kernel(**inputs):
    inputs = {k: np.asarray(v) for k, v in inputs.items()}
    res = _run(inputs)
    out = np.empty((B, 1), np.float32)
    for cid in range(NCORES):
        out[2 * cid, 0] = res.results[cid]["odec"][0, 0]
        out[2 * cid + 1, 0] = res.results[cid]["odec"][0, 1]
    return out


# revision 41
# speedup vs baseline: 1.1103x; 1.0210x over previous
"""MiniS4D Trainium2 kernel — channel-sharded SSM + AllToAll + batch-parallel mix.

Sharding: each of the 8 cores computes the (channel-independent) SSM /
depthwise-conv / GELU for its 64 channels across ALL 16 batches, with fat
512-column matmuls.  The GELU output y is resharded with two AllToAll
collectives (split by batch parity, the second overlapped with the mix) so
each core then holds all 512 channels for 2 batches, and runs the pointwise
channel-mix GEMM + GLU + mean + decode locally.

Math (per channel): conv with the TIME-REVERSED S4D kernel, decomposed into
chunks of T=128 (M=32 chunks):
  intra (lags 0..127): Toeplitz matmul with keff[0:128] (+D at lag 0);
    flipped operands: stationary = u-chunk [t, (4b x 32m)], moving = toep
    [t_in, t_out] so PSUM comes out [(b,m), t] — DMA-friendly, no transposes.
  inter (lags >= 128): rank-16 state expansion;
    Q[n,(b,m)] = sum_t r^t u[128m+t]   (B1 matmuls, 512 cols each)
    H[s] = prefix-sum over m<s of e^m Q[m], e = r^128   (DVE+Pool scan)
    G[s] = r^(L-1-128s) H[s]                            (postscale)
    y_inter[(b,m),t] = G-stationary x v1-moving matmul accumulated into the
    same PSUM tile as intra.

The mix GEMM runs in fp8-e4m3 DoubleRow mode (K=256 per pass) when FP8 is
set; y travels through the AllToAll in fp8 (half the wire bytes).  fp16
operands elsewhere, fp32 accumulation.  Output (16,1) assembled on host
from per-core (1,2); global batch b = 2*cid + bl.
"""
import sys
sys.path.insert(0, "/opt/trn_rl_repo")
import numpy as np

import concourse.bass as bass
import concourse.tile as tile
from concourse import bacc, mybir
from concourse import bass_utils

F32 = mybir.dt.float32
F16 = mybir.dt.float16
F8 = mybir.dt.float8e4
AF = mybir.ActivationFunctionType
ALU = mybir.AluOpType

FP8 = True              # fp8 y + fp8 DoubleRow mix

B, C, L, N = 16, 512, 4096, 8
T, M = 128, 32          # chunk length, number of chunks
S = 32                  # inter-state slots (slot s = chunk s), slot 0 == 0
NCORES = 8
CS = C // NCORES        # 64 channels per core
BL = 2                  # batches per core in the mix phase
NW = CS // 4            # 16 B1 waves (4 channels each: c = 4w + q)

# b' -> global batch permutation: even batches first (a2a chunk 0), then odd.
BPERM = np.r_[np.arange(0, B, 2), np.arange(1, B, 2)]

# swap re<->im rows within each 16-row half of every 32-partition group
SHUF32 = [(i // 16) * 16 + ((i % 16) + 8) % 16 for i in range(32)]

_compiled = None


def _prep(inputs):
    """Host-side parameter preparation (numpy, float64 internally)."""
    log_dt = inputs["log_dt"].astype(np.float64)
    A = -np.exp(inputs["log_A_real"].astype(np.float64)) \
        + 1j * inputs["A_imag"].astype(np.float64)            # (C, N)
    dt = np.exp(log_dt)
    r = np.exp(dt[:, None] * A)                                # (C, N)
    Bc = inputs["B_re"].astype(np.float64) + 1j * inputs["B_im"].astype(np.float64)
    Cc = inputs["C_re"].astype(np.float64) + 1j * inputs["C_im"].astype(np.float64)
    wv = Cc * (r - 1.0) / A * Bc                               # (C, N)
    rinv = 1.0 / r
    lags = np.arange(T)
    wL = wv * r ** (L - 1)
    keff = np.real(wL[:, :, None] * rinv[:, :, None] ** lags).sum(1)   # (C, T)
    keff[:, 0] += inputs["D"].astype(np.float64)

    toep = np.zeros((C, T, T), np.float16)
    for d in range(T):
        idx = np.arange(T - d)
        toep[:, idx, idx + d] = keff[:, d].astype(np.float16)[:, None]

    pw = r[:, :, None] ** lags                                 # (C, N, T)
    v2 = np.concatenate([pw.real, pw.imag], 1)                 # (C, 16, T)
    v2s = np.concatenate([pw.imag, pw.real], 1)                # row-swapped
    pw1 = wv[:, :, None] * rinv[:, :, None] ** lags
    v1 = np.concatenate([pw1.real, -pw1.imag], 1)              # (C, 16, T)

    e = r ** T                                                 # (C, N)
    Epow = e[:, :, None] ** np.arange(S - 1)                   # (C, N, 31): e^j
    Kp = r[:, :, None] ** (L - 1 - T * np.arange(S))           # (C, N, 32)
    Kp[:, :, 0] = 0.0

    # W is pre-scaled by WSC so fp8 quantization stays in the normal range;
    # 1/WSC is folded into the sigmoid scale and the decode weights.
    WSC = 1.0
    WT = np.ascontiguousarray(inputs["W_out"].T * WSC).astype(np.float16)
    # [ct2, p, i, o] with c = 256*ct2 + 128*i + p (DoubleRow k-pair layout)
    wmix8 = np.ascontiguousarray(
        WT.reshape(2, 2, 128, 1024).transpose(0, 2, 1, 3))
    b_out = inputs["b_out"].astype(np.float32)
    bouta = np.ascontiguousarray(b_out[:512].reshape(4, 128).T) * WSC   # (128, 4)
    boutg = np.ascontiguousarray(b_out[512:].reshape(4, 128).T)
    wd = (inputs["W_dec"][0].astype(np.float32) / (L * WSC)).reshape(4, 128).T
    wdec = np.ascontiguousarray(np.repeat(wd[:, None, :], BL, axis=1))  # (128, 2, 4)
    bdec = inputs["b_dec"].astype(np.float32).reshape(1, 1)

    u16 = inputs["u"].astype(np.float16)                                # (B, C, L)

    in_maps = []
    for cid in range(NCORES):
        c0 = CS * cid
        cs = slice(c0, c0 + CS)
        # uT[t, c, b', m] = u[BPERM[b'], c0+c, 128m+t]
        uT = np.ascontiguousarray(
            u16[BPERM, cs].reshape(B, CS, M, T).transpose(3, 1, 0, 2))
        toep2 = np.ascontiguousarray(toep[cs].transpose(1, 0, 2))       # [i, c, j]
        v2t = np.ascontiguousarray(
            v2[cs].transpose(2, 0, 1).astype(np.float16))               # [t, c, nh]
        v2ts = np.ascontiguousarray(
            v2s[cs].transpose(2, 0, 1).astype(np.float16))
        v1mt = np.zeros((128, NW, T), np.float16)
        e1h = np.zeros((128, NW, S - 1), np.float64)
        e2h = np.zeros((128, NW, S - 1), np.float64)
        k1h = np.zeros((128, NW, S), np.float64)
        k2h = np.zeros((128, NW, S), np.float64)
        for q in range(4):
            cq = c0 + 4 * np.arange(NW) + q                             # w -> channel
            v1mt[32 * q:32 * q + 16] = v1[cq].transpose(1, 0, 2)        # [16nh, w, t]
            Eq = Epow[cq].transpose(1, 0, 2)                            # (8n, w, 31)
            e1h[32 * q:32 * q + 8] = Eq.real
            e1h[32 * q + 8:32 * q + 16] = Eq.real
            e2h[32 * q:32 * q + 8] = -Eq.imag
            e2h[32 * q + 8:32 * q + 16] = Eq.imag
            Kq = Kp[cq].transpose(1, 0, 2)                              # (8n, w, 32)
            k1h[32 * q:32 * q + 8] = Kq.real
            k1h[32 * q + 8:32 * q + 16] = Kq.real
            k2h[32 * q:32 * q + 8] = -Kq.imag
            k2h[32 * q + 8:32 * q + 16] = Kq.imag
        in_maps.append(dict(
            uT=uT, toep2=toep2, v2t=v2t, v2ts=v2ts, v1mt=v1mt,
            e1h=e1h.astype(np.float16), e2h=e2h.astype(np.float16),
            k1h=k1h.astype(np.float16), k2h=k2h.astype(np.float16),
            wmix8=wmix8, bouta=bouta, boutg=boutg, wdec=wdec, bdec=bdec,
        ))
    return in_maps


def _build():
    nc = bacc.Bacc("TRN2", target_bir_lowering=False, debug=False,
                   num_devices=NCORES)
    d_uT = nc.dram_tensor("uT", [T, CS, B, M], F16, kind="ExternalInput").ap()
    d_toep = nc.dram_tensor("toep2", [T, CS, T], F16, kind="ExternalInput").ap()
    d_v2t = nc.dram_tensor("v2t", [T, CS, 16], F16, kind="ExternalInput").ap()
    d_v2ts = nc.dram_tensor("v2ts", [T, CS, 16], F16, kind="ExternalInput").ap()
    d_v1mt = nc.dram_tensor("v1mt", [128, NW, T], F16, kind="ExternalInput").ap()
    d_e1h = nc.dram_tensor("e1h", [128, NW, S - 1], F16, kind="ExternalInput").ap()
    d_e2h = nc.dram_tensor("e2h", [128, NW, S - 1], F16, kind="ExternalInput").ap()
    d_k1h = nc.dram_tensor("k1h", [128, NW, S], F16, kind="ExternalInput").ap()
    d_k2h = nc.dram_tensor("k2h", [128, NW, S], F16, kind="ExternalInput").ap()
    d_wmix8 = nc.dram_tensor("wmix8", [2, 128, 2, 1024], F16,
                             kind="ExternalInput").ap()
    d_bouta = nc.dram_tensor("bouta", [128, 4], F32, kind="ExternalInput").ap()
    d_boutg = nc.dram_tensor("boutg", [128, 4], F32, kind="ExternalInput").ap()
    d_wdec = nc.dram_tensor("wdec", [128, BL, 4], F32, kind="ExternalInput").ap()
    d_bdec = nc.dram_tensor("bdec", [1, 1], F32, kind="ExternalInput").ap()
    d_out = nc.dram_tensor("odec", [1, BL], F32, kind="ExternalOutput").ap()

    FY = F8 if FP8 else F16

    with tile.TileContext(nc) as tc:
        with tc.tile_pool(name="dram", bufs=1, space="DRAM") as dram, \
             tc.tile_pool(name="const", bufs=1) as constp:
            # a2a bounce buffers: par p holds batches of parity p
            # yin[bp, m, c, t] = y[c0+c, 2*bp + par, 128m + t]
            d_yin = [dram.tile([NCORES, M, CS, T], FY, tag=f"yin{p}",
                               name=f"yin{p}") for p in range(2)]
            d_yc = [dram.tile([NCORES, M, CS, T], FY, tag=f"yc{p}",
                              name=f"yc{p}") for p in range(2)]

            # small params first so B1 can start almost immediately
            bouta_sb = constp.tile([128, 4], F32)
            nc.scalar.dma_start(bouta_sb[:], d_bouta[:])
            boutg_sb = constp.tile([128, 4], F32)
            nc.scalar.dma_start(boutg_sb[:], d_boutg[:])
            wdec_sb = constp.tile([128, BL, 4], F32)
            nc.scalar.dma_start(wdec_sb[:], d_wdec[:])
            bdec_sb = constp.tile([1, 1], F32)
            nc.scalar.dma_start(bdec_sb[:], d_bdec[:])

            # ================= SSM phase (64 channels, 16 batches) ========
            with tc.tile_pool(name="uTp", bufs=1) as uTp, \
                 tc.tile_pool(name="prm", bufs=1) as prm, \
                 tc.tile_pool(name="Hp", bufs=1) as Hp:
                v2t = prm.tile([T, CS, 16], F16)
                nc.scalar.dma_start(v2t[:], d_v2t[:])
                v2ts = prm.tile([T, CS, 16], F16)
                nc.scalar.dma_start(v2ts[:], d_v2ts[:])
                scanmask = prm.tile([128, NW, B, S], F16)
                nc.vector.memset(scanmask[:], 1.0)
                nc.vector.memset(scanmask[:, :, :, 0:1], 0.0)
                e1h = prm.tile([128, NW, S - 1], F16)
                nc.scalar.dma_start(e1h[:], d_e1h[:])
                e2h = prm.tile([128, NW, S - 1], F16)
                nc.scalar.dma_start(e2h[:], d_e2h[:])
                k1h = prm.tile([128, NW, S], F16)
                nc.scalar.dma_start(k1h[:], d_k1h[:])
                k2h = prm.tile([128, NW, S], F16)
                nc.scalar.dma_start(k2h[:], d_k2h[:])
                v1mt = prm.tile([128, NW, T], F16)
                nc.scalar.dma_start(v1mt[:], d_v1mt[:])
                uT = uTp.tile([T, CS, B, M], F16)
                for ck in range(4):
                    sl = slice(16 * ck, 16 * ck + 16)
                    nc.sync.dma_start(uT[:, sl], d_uT[:, sl])
                toepT = prm.tile([T, CS, T], F16)
                for ck in range(4):
                    sl = slice(16 * ck, 16 * ck + 16)
                    nc.scalar.dma_start(toepT[:, sl], d_toep[:, sl])
                # mix weights: DMA now, but cast to fp8 only after the SSM
                # phase — a cast issued here would head-block the DVE queue
                # (and thus all of B1's prescale) on the 4 MiB weight DMA.
                wm = []
                wtmps = []
                for ct2 in range(2):
                    wtmp = constp.tile([128, 2, 1024], F16, name=f"wtmp{ct2}")
                    nc.scalar.dma_start(wtmp[:], d_wmix8[ct2])
                    wtmps.append(wtmp)
                    if FP8:
                        w8 = constp.tile([128, 2, 1024], F8, name=f"w8{ct2}")
                        wm.append(w8)
                    else:
                        wm.append(wtmp)

                H = Hp.tile([128, NW, B, S], F16)
                nc.vector.memset(H[:, :, :, 0:1], 0.0)

                # ---- B1: state matmuls + prescale (Q -> e^m Q) ----
                # The PE also emits a re<->im swapped copy of Q (second
                # stationary) so the complex prescale needs no DVE shuffle.
                with tc.tile_pool(name="hps", bufs=3, space="PSUM") as hps, \
                     tc.tile_pool(name="b1s", bufs=3) as b1s:
                    for w in range(NW):
                        hb = hps.tile([128, B, M], F32, tag="hb")
                        hbs = hps.tile([128, B, M], F32, tag="hbs")
                        for q in range(4):
                            c = 4 * w + q
                            nc.tensor.matmul(
                                hb[32 * q:32 * q + 16, :, :],
                                v2t[:, c, :], uT[:, c, :, :],
                                start=True, stop=True,
                                tile_position=(0, 32 * q))
                            nc.tensor.matmul(
                                hbs[32 * q:32 * q + 16, :, :],
                                v2ts[:, c, :], uT[:, c, :, :],
                                start=True, stop=True,
                                tile_position=(0, 32 * q))
                        t1 = b1s.tile([128, B, S - 1], F16, tag="t1")
                        e1b = e1h[:, w].unsqueeze(1).broadcast_to([128, B, S - 1])
                        e2b = e2h[:, w].unsqueeze(1).broadcast_to([128, B, S - 1])
                        nc.vector.tensor_mul(t1[:], hb[:, :, 0:S - 1], e1b)
                        nc.vector.tensor_mul(hbs[:, :, 0:S - 1],
                                             hbs[:, :, 0:S - 1], e2b)
                        nc.vector.tensor_add(H[:, w, :, 1:S], t1[:],
                                             hbs[:, :, 0:S - 1])

                # ---- scan over chunks: one fused prefix-sum, with the
                # mask resetting the carry at each (w, b) group boundary ----
                nc.vector.tensor_tensor_scan(
                    H[:].rearrange("p a b c -> p (a b c)"),
                    scanmask[:].rearrange("p a b c -> p (a b c)"),
                    H[:].rearrange("p a b c -> p (a b c)"),
                    0.0, op0=ALU.mult, op1=ALU.add)

                # ---- postscale: G = K * H (4 w-groups, DVE+Pool split) ----
                with tc.tile_pool(name="pss", bufs=2) as pss:
                    for wg in range(4):
                        wsl = slice(4 * wg, 4 * wg + 4)
                        sw2 = pss.tile([128, 4, B, S], F16, tag="sw2")
                        t2 = pss.tile([128, 4, B, S], F16, tag="t2")
                        hgc = H[:, wsl]
                        k1b = k1h[:, wsl].unsqueeze(2).broadcast_to([128, 4, B, S])
                        k2b = k2h[:, wsl].unsqueeze(2).broadcast_to([128, 4, B, S])
                        nc.vector.stream_shuffle(sw2[:], hgc, SHUF32)
                        nc.vector.tensor_mul(t2[:], hgc, k1b)
                        nc.gpsimd.tensor_mul(sw2[:], sw2[:], k2b)
                        nc.vector.tensor_add(hgc, t2[:], sw2[:])

                # ---- B2: intra Toeplitz + inter expand + GELU -> y_in ----
                # PSUM out [(4b' x 32m), t] per (channel, batch-group);
                # intra x4 then inter x4 so LDWEIGHTS pipelines with moving.
                with tc.tile_pool(name="yps", bufs=6, space="PSUM") as ypsp, \
                     tc.tile_pool(name="stg", bufs=4) as stgp:
                    for g in range(4):          # b' 4g..4g+4; parity par=g//2
                        par = g // 2
                        bsl = slice(4 * g, 4 * g + 4)
                        for cw in range(NW):
                            yps = ypsp.tile([128, 4, T], F32)
                            for q in range(4):
                                c = 4 * cw + q
                                nc.tensor.matmul(
                                    yps[:, q, :],
                                    uT[:, c, bsl, :], toepT[:, c, :],
                                    start=True, stop=False)
                                nc.tensor.matmul(
                                    yps[:, q, :],
                                    H[32 * q:32 * q + 16, cw, bsl, 0:S],
                                    v1mt[32 * q:32 * q + 16, cw, :],
                                    start=False, stop=True,
                                    tile_position=(32 * q, 0))
                            st = stgp.tile([128, 4, T], FY)
                            nc.scalar.activation(st[:], yps[:], AF.Gelu)
                            # dst [bp(4), m, c(4), t] <- src [(4b',32m),(4c,t)]
                            bp0 = 4 * (g % 2)
                            nc.sync.dma_start(
                                d_yin[par][bp0:bp0 + 4, :, 4 * cw:4 * cw + 4],
                                st[:])
                        if g == 1 or g == 3:
                            nc.gpsimd.collective_compute(
                                "AllToAll",
                                mybir.AluOpType.bypass,
                                replica_groups=[list(range(NCORES))],
                                ins=[d_yin[par][:].opt()],
                                outs=[d_yc[par][:].opt()],
                            )

            # ================= Mix phase (2 batches, 512 channels) ========
            if FP8:
                for ct2 in range(2):
                    nc.vector.tensor_copy(wm[ct2][:], wtmps[ct2][:])
            with tc.tile_pool(name="ytp", bufs=1) as ytp, \
                 tc.tile_pool(name="sgp", bufs=4) as sgp, \
                 tc.tile_pool(name="m1p", bufs=1) as m1p:
                M1 = m1p.tile([128, BL, 4, 8], F32)
                # prefetch y tiles on the gpsimd queue: it is empty after the
                # (non-blocking) a2a triggers, so the loads dispatch the
                # moment each collective's completion semaphore fires, and
                # their waits cannot head-block GELUs or sigmoids.
                ytF2 = {}
                for bl in range(BL):
                    eng = nc.gpsimd
                    for ct2 in range(2):
                        t = ytp.tile([128, 2, M, T], FY, name=f"yt{bl}{ct2}")
                        for i in range(2):
                            for h in range(2):
                                eng.dma_start(
                                    t[64 * h:64 * h + 64, i],
                                    d_yc[bl][4 * ct2 + 2 * i + h]
                                    .transpose([1, 0, 2]))
                        ytF2[(bl, ct2)] = t
                with tc.tile_pool(name="zps", bufs=2, space="PSUM") as zpsp:
                    for bl in range(BL):
                        for pr in range(4):
                            for pp in range(4):
                                lcs = (2 * pp, 2 * pp + 1)
                                za2 = zpsp.tile([128, 2, 512], F32, tag="za")
                                zg2 = zpsp.tile([128, 2, 512], F32, tag="zg")
                                for side, zt in ((0, za2), (1, zg2)):
                                    ot = pr + 4 * side
                                    osl = slice(128 * ot, 128 * ot + 128)
                                    for ih, lc in enumerate(lcs):
                                        csl = slice(4 * lc, 4 * lc + 4)
                                        if FP8:
                                            for ct2 in range(2):
                                                nc.tensor.matmul(
                                                    zt[:, ih],
                                                    wm[ct2][:, :, osl],
                                                    ytF2[(bl, ct2)][:, :, csl, :]
                                                    .rearrange("p i a b -> p i (a b)"),
                                                    start=(ct2 == 0),
                                                    stop=(ct2 == 1),
                                                    perf_mode=mybir.MatmulPerfMode.DoubleRow)
                                        else:
                                            for ct2 in range(2):
                                                for i in range(2):
                                                    nc.tensor.matmul(
                                                        zt[:, ih],
                                                        wm[ct2][:, i, osl],
                                                        ytF2[(bl, ct2)][:, i, csl, :]
                                                        .rearrange("p a b -> p (a b)"),
                                                        start=(ct2 == 0 and i == 0),
                                                        stop=(ct2 == 1 and i == 1))
                                sg2 = sgp.tile([128, 2, 512], F16, tag="sg")
                                nc.scalar.activation(
                                    sg2[:], zg2[:], AF.Sigmoid,
                                    bias=boutg_sb[:, pr:pr + 1],
                                    scale=1.0)
                                for ih, lc in enumerate(lcs):
                                    scr = sgp.tile([128, 512], F16, tag="scr")
                                    nc.vector.scalar_tensor_tensor(
                                        scr[:], za2[:, ih],
                                        bouta_sb[:, pr:pr + 1],
                                        sg2[:, ih],
                                        op0=ALU.add, op1=ALU.mult,
                                        accum_out=M1[:, bl:bl + 1, pr:pr + 1,
                                                     lc:lc + 1].squeeze()
                                        .unsqueeze(1))

                # ---- decode ----
                with tc.tile_pool(name="dps", bufs=1, space="PSUM") as dpsp:
                    R1 = m1p.tile([128, BL, 4], F32)
                    nc.vector.reduce_sum(R1[:], M1[:], axis=mybir.AxisListType.X)
                    R2 = m1p.tile([128, BL, 4], F32)
                    nc.vector.tensor_mul(R2[:], R1[:], wdec_sb[:])
                    R3 = m1p.tile([128, BL], F32)
                    nc.vector.reduce_sum(R3[:], R2[:], axis=mybir.AxisListType.X)
                    ones = m1p.tile([128, 1], F32)
                    nc.vector.memset(ones[:], 1.0)
                    dp = dpsp.tile([1, BL], F32)
                    nc.tensor.matmul(dp[:], ones[:], R3[:], start=True, stop=True)
                    osb = m1p.tile([1, BL], F32)
                    nc.vector.tensor_scalar_add(osb[:], dp[:], bdec_sb[:, 0:1])
                    nc.sync.dma_start(d_out[:], osb[:])

    nc.compile()
    return nc


def _get_compiled():
    global _compiled
    if _compiled is None:
        _compiled = _build()
    return _compiled


def _run(inputs, trace=False, **kw):
    in_maps = _prep(inputs)
    nc = _get_compiled()
    return bass_utils.run_bass_kernel_spmd(
        nc, in_maps, core_ids=list(range(NCORES)), trace=trace, **kw)


def kernel(**inputs):
    inputs = {k: np.asarray(v) for k, v in inputs.items()}
    res = _run(inputs)
    out = np.empty((B, 1), np.float32)
    for cid in range(NCORES):
        out[2 * cid, 0] = res.results[cid]["odec"][0, 0]
        out[2 * cid + 1, 0] = res.results[cid]["odec"][0, 1]
    return out


# revision 42
# speedup vs baseline: 1.1329x; 1.0203x over previous
"""MiniS4D Trainium2 kernel — channel-sharded SSM + AllToAll + batch-parallel mix.

Sharding: each of the 8 cores computes the (channel-independent) SSM /
depthwise-conv / GELU for its 64 channels across ALL 16 batches, with fat
512-column matmuls.  The GELU output y is resharded with two AllToAll
collectives (split by batch parity, the second overlapped with the mix) so
each core then holds all 512 channels for 2 batches, and runs the pointwise
channel-mix GEMM + GLU + mean + decode locally.

Math (per channel): conv with the TIME-REVERSED S4D kernel, decomposed into
chunks of T=128 (M=32 chunks):
  intra (lags 0..127): Toeplitz matmul with keff[0:128] (+D at lag 0);
    flipped operands: stationary = u-chunk [t, (4b x 32m)], moving = toep
    [t_in, t_out] so PSUM comes out [(b,m), t] — DMA-friendly, no transposes.
  inter (lags >= 128): rank-16 state expansion;
    Q[n,(b,m)] = sum_t r^t u[128m+t]   (B1 matmuls, 512 cols each)
    H[s] = prefix-sum over m<s of e^m Q[m], e = r^128   (DVE+Pool scan)
    G[s] = r^(L-1-128s) H[s]                            (postscale)
    y_inter[(b,m),t] = G-stationary x v1-moving matmul accumulated into the
    same PSUM tile as intra.

The mix GEMM runs in fp8-e4m3 DoubleRow mode (K=256 per pass) when FP8 is
set; y travels through the AllToAll in fp8 (half the wire bytes).  fp16
operands elsewhere, fp32 accumulation.  Output (16,1) assembled on host
from per-core (1,2); global batch b = 2*cid + bl.
"""
import sys
sys.path.insert(0, "/opt/trn_rl_repo")
import numpy as np

import concourse.bass as bass
import concourse.tile as tile
from concourse import bacc, mybir
from concourse import bass_utils

F32 = mybir.dt.float32
F16 = mybir.dt.float16
F8 = mybir.dt.float8e4
AF = mybir.ActivationFunctionType
ALU = mybir.AluOpType

FP8 = True              # fp8 y + fp8 DoubleRow mix

B, C, L, N = 16, 512, 4096, 8
T, M = 128, 32          # chunk length, number of chunks
S = 32                  # inter-state slots (slot s = chunk s), slot 0 == 0
NCORES = 8
CS = C // NCORES        # 64 channels per core
BL = 2                  # batches per core in the mix phase
NW = CS // 4            # 16 B1 waves (4 channels each: c = 4w + q)

# b' -> global batch permutation: even batches first (a2a chunk 0), then odd.
BPERM = np.r_[np.arange(0, B, 2), np.arange(1, B, 2)]

# swap re<->im rows within each 16-row half of every 32-partition group
SHUF32 = [(i // 16) * 16 + ((i % 16) + 8) % 16 for i in range(32)]

_compiled = None


def _prep(inputs):
    """Host-side parameter preparation (numpy, float64 internally)."""
    log_dt = inputs["log_dt"].astype(np.float64)
    A = -np.exp(inputs["log_A_real"].astype(np.float64)) \
        + 1j * inputs["A_imag"].astype(np.float64)            # (C, N)
    dt = np.exp(log_dt)
    r = np.exp(dt[:, None] * A)                                # (C, N)
    Bc = inputs["B_re"].astype(np.float64) + 1j * inputs["B_im"].astype(np.float64)
    Cc = inputs["C_re"].astype(np.float64) + 1j * inputs["C_im"].astype(np.float64)
    wv = Cc * (r - 1.0) / A * Bc                               # (C, N)
    rinv = 1.0 / r
    lags = np.arange(T)
    wL = wv * r ** (L - 1)
    keff = np.real(wL[:, :, None] * rinv[:, :, None] ** lags).sum(1)   # (C, T)
    keff[:, 0] += inputs["D"].astype(np.float64)

    toep = np.zeros((C, T, T), np.float16)
    for d in range(T):
        idx = np.arange(T - d)
        toep[:, idx, idx + d] = keff[:, d].astype(np.float16)[:, None]

    pw = r[:, :, None] ** lags                                 # (C, N, T)
    v2 = np.concatenate([pw.real, pw.imag], 1)                 # (C, 16, T)
    v2s = np.concatenate([pw.imag, pw.real], 1)                # row-swapped
    pw1 = wv[:, :, None] * rinv[:, :, None] ** lags
    v1 = np.concatenate([pw1.real, -pw1.imag], 1)              # (C, 16, T)

    e = r ** T                                                 # (C, N)
    Epow = e[:, :, None] ** np.arange(S - 1)                   # (C, N, 31): e^j
    Kp = r[:, :, None] ** (L - 1 - T * np.arange(S))           # (C, N, 32)
    Kp[:, :, 0] = 0.0

    # W is pre-scaled by WSC so fp8 quantization stays in the normal range;
    # 1/WSC is folded into the sigmoid scale and the decode weights.
    WSC = 1.0
    WT = np.ascontiguousarray(inputs["W_out"].T * WSC).astype(np.float16)
    # [ct2, p, i, o] with c = 256*ct2 + 128*i + p (DoubleRow k-pair layout)
    wmix8 = np.ascontiguousarray(
        WT.reshape(2, 2, 128, 1024).transpose(0, 2, 1, 3))
    b_out = inputs["b_out"].astype(np.float32)
    bouta = np.ascontiguousarray(b_out[:512].reshape(4, 128).T) * WSC   # (128, 4)
    boutg = np.ascontiguousarray(b_out[512:].reshape(4, 128).T)
    wd = (inputs["W_dec"][0].astype(np.float32) / (L * WSC)).reshape(4, 128).T
    wdec = np.ascontiguousarray(np.repeat(wd[:, None, :], BL, axis=1))  # (128, 2, 4)
    bdec = inputs["b_dec"].astype(np.float32).reshape(1, 1)

    u16 = inputs["u"].astype(np.float16)                                # (B, C, L)

    in_maps = []
    for cid in range(NCORES):
        c0 = CS * cid
        cs = slice(c0, c0 + CS)
        # uT[t, c, b', m] = u[BPERM[b'], c0+c, 128m+t]
        uT = np.ascontiguousarray(
            u16[BPERM, cs].reshape(B, CS, M, T).transpose(3, 1, 0, 2))
        toep2 = np.ascontiguousarray(toep[cs].transpose(1, 0, 2))       # [i, c, j]
        v2t = np.ascontiguousarray(
            v2[cs].transpose(2, 0, 1).astype(np.float16))               # [t, c, nh]
        v2ts = np.ascontiguousarray(
            v2s[cs].transpose(2, 0, 1).astype(np.float16))
        v1mt = np.zeros((128, NW, T), np.float16)
        e1h = np.zeros((128, NW, S - 1), np.float64)
        e2h = np.zeros((128, NW, S - 1), np.float64)
        k1h = np.zeros((128, NW, S), np.float64)
        k2h = np.zeros((128, NW, S), np.float64)
        for q in range(4):
            cq = c0 + 4 * np.arange(NW) + q                             # w -> channel
            v1mt[32 * q:32 * q + 16] = v1[cq].transpose(1, 0, 2)        # [16nh, w, t]
            Eq = Epow[cq].transpose(1, 0, 2)                            # (8n, w, 31)
            e1h[32 * q:32 * q + 8] = Eq.real
            e1h[32 * q + 8:32 * q + 16] = Eq.real
            e2h[32 * q:32 * q + 8] = -Eq.imag
            e2h[32 * q + 8:32 * q + 16] = Eq.imag
            Kq = Kp[cq].transpose(1, 0, 2)                              # (8n, w, 32)
            k1h[32 * q:32 * q + 8] = Kq.real
            k1h[32 * q + 8:32 * q + 16] = Kq.real
            k2h[32 * q:32 * q + 8] = -Kq.imag
            k2h[32 * q + 8:32 * q + 16] = Kq.imag
        in_maps.append(dict(
            uT=uT, toep2=toep2, v2t=v2t, v2ts=v2ts, v1mt=v1mt,
            e1h=e1h.astype(np.float16), e2h=e2h.astype(np.float16),
            k1h=k1h.astype(np.float16), k2h=k2h.astype(np.float16),
            wmix8=wmix8, bouta=bouta, boutg=boutg, wdec=wdec, bdec=bdec,
        ))
    return in_maps


def _build():
    nc = bacc.Bacc("TRN2", target_bir_lowering=False, debug=False,
                   num_devices=NCORES)
    d_uT = nc.dram_tensor("uT", [T, CS, B, M], F16, kind="ExternalInput").ap()
    d_toep = nc.dram_tensor("toep2", [T, CS, T], F16, kind="ExternalInput").ap()
    d_v2t = nc.dram_tensor("v2t", [T, CS, 16], F16, kind="ExternalInput").ap()
    d_v2ts = nc.dram_tensor("v2ts", [T, CS, 16], F16, kind="ExternalInput").ap()
    d_v1mt = nc.dram_tensor("v1mt", [128, NW, T], F16, kind="ExternalInput").ap()
    d_e1h = nc.dram_tensor("e1h", [128, NW, S - 1], F16, kind="ExternalInput").ap()
    d_e2h = nc.dram_tensor("e2h", [128, NW, S - 1], F16, kind="ExternalInput").ap()
    d_k1h = nc.dram_tensor("k1h", [128, NW, S], F16, kind="ExternalInput").ap()
    d_k2h = nc.dram_tensor("k2h", [128, NW, S], F16, kind="ExternalInput").ap()
    d_wmix8 = nc.dram_tensor("wmix8", [2, 128, 2, 1024], F16,
                             kind="ExternalInput").ap()
    d_bouta = nc.dram_tensor("bouta", [128, 4], F32, kind="ExternalInput").ap()
    d_boutg = nc.dram_tensor("boutg", [128, 4], F32, kind="ExternalInput").ap()
    d_wdec = nc.dram_tensor("wdec", [128, BL, 4], F32, kind="ExternalInput").ap()
    d_bdec = nc.dram_tensor("bdec", [1, 1], F32, kind="ExternalInput").ap()
    d_out = nc.dram_tensor("odec", [1, BL], F32, kind="ExternalOutput").ap()

    FY = F8 if FP8 else F16

    with tile.TileContext(nc) as tc:
        with tc.tile_pool(name="dram", bufs=1, space="DRAM") as dram, \
             tc.tile_pool(name="const", bufs=1) as constp:
            # a2a bounce buffers: par p holds batches of parity p
            # yin[bp, m, c, t] = y[c0+c, 2*bp + par, 128m + t]
            d_yin = [dram.tile([NCORES, M, CS, T], FY, tag=f"yin{p}",
                               name=f"yin{p}") for p in range(2)]
            d_yc = [dram.tile([NCORES, M, CS, T], FY, tag=f"yc{p}",
                              name=f"yc{p}") for p in range(2)]

            # small params first so B1 can start almost immediately
            bouta_sb = constp.tile([128, 4], F32)
            nc.scalar.dma_start(bouta_sb[:], d_bouta[:])
            boutg_sb = constp.tile([128, 4], F32)
            nc.scalar.dma_start(boutg_sb[:], d_boutg[:])
            wdec_sb = constp.tile([128, BL, 4], F32)
            nc.scalar.dma_start(wdec_sb[:], d_wdec[:])
            bdec_sb = constp.tile([1, 1], F32)
            nc.scalar.dma_start(bdec_sb[:], d_bdec[:])

            # ================= SSM phase (64 channels, 16 batches) ========
            with tc.tile_pool(name="uTp", bufs=1) as uTp, \
                 tc.tile_pool(name="prm", bufs=1) as prm, \
                 tc.tile_pool(name="Hp", bufs=1) as Hp:
                v2t = prm.tile([T, CS, 16], F16)
                nc.scalar.dma_start(v2t[:], d_v2t[:])
                v2ts = prm.tile([T, CS, 16], F16)
                nc.scalar.dma_start(v2ts[:], d_v2ts[:])
                scanmask = prm.tile([128, NW, B, S], F16)
                nc.vector.memset(scanmask[:], 1.0)
                nc.vector.memset(scanmask[:, :, :, 0:1], 0.0)
                e1h = prm.tile([128, NW, S - 1], F16)
                nc.scalar.dma_start(e1h[:], d_e1h[:])
                e2h = prm.tile([128, NW, S - 1], F16)
                nc.scalar.dma_start(e2h[:], d_e2h[:])
                k1h = prm.tile([128, NW, S], F16)
                nc.scalar.dma_start(k1h[:], d_k1h[:])
                k2h = prm.tile([128, NW, S], F16)
                nc.scalar.dma_start(k2h[:], d_k2h[:])
                v1mt = prm.tile([128, NW, T], F16)
                nc.scalar.dma_start(v1mt[:], d_v1mt[:])
                uT = uTp.tile([T, CS, B, M], F16)
                for ck in range(4):
                    sl = slice(16 * ck, 16 * ck + 16)
                    nc.sync.dma_start(uT[:, sl], d_uT[:, sl])
                toepT = prm.tile([T, CS, T], F16)
                for ck in range(4):
                    sl = slice(16 * ck, 16 * ck + 16)
                    nc.scalar.dma_start(toepT[:, sl], d_toep[:, sl])
                # mix weights: DMA now, but cast to fp8 only after the SSM
                # phase — a cast issued here would head-block the DVE queue
                # (and thus all of B1's prescale) on the 4 MiB weight DMA.
                wm = []
                wtmps = []
                for ct2 in range(2):
                    wtmp = constp.tile([128, 2, 1024], F16, name=f"wtmp{ct2}")
                    nc.scalar.dma_start(wtmp[:], d_wmix8[ct2])
                    wtmps.append(wtmp)
                    if FP8:
                        w8 = constp.tile([128, 2, 1024], F8, name=f"w8{ct2}")
                        wm.append(w8)
                    else:
                        wm.append(wtmp)

                H = Hp.tile([128, NW, B, S], F16)
                nc.vector.memset(H[:, :, :, 0:1], 0.0)

                # ---- B1: state matmuls + prescale (Q -> e^m Q) ----
                # The PE also emits a re<->im swapped copy of Q (second
                # stationary) so the complex prescale needs no DVE shuffle.
                with tc.tile_pool(name="hps", bufs=4, space="PSUM") as hps, \
                     tc.tile_pool(name="b1s", bufs=4) as b1s:
                    for w in range(NW):
                        hb = hps.tile([128, B, M], F32, tag="hb")
                        hbs = hps.tile([128, B, M], F32, tag="hbs")
                        for q in range(4):
                            c = 4 * w + q
                            nc.tensor.matmul(
                                hb[32 * q:32 * q + 16, :, :],
                                v2t[:, c, :], uT[:, c, :, :],
                                start=True, stop=True,
                                tile_position=(0, 32 * q))
                            nc.tensor.matmul(
                                hbs[32 * q:32 * q + 16, :, :],
                                v2ts[:, c, :], uT[:, c, :, :],
                                start=True, stop=True,
                                tile_position=(0, 32 * q))
                        t1 = b1s.tile([128, B, S - 1], F16, tag="t1")
                        e1b = e1h[:, w].unsqueeze(1).broadcast_to([128, B, S - 1])
                        e2b = e2h[:, w].unsqueeze(1).broadcast_to([128, B, S - 1])
                        nc.vector.tensor_mul(t1[:], hb[:, :, 0:S - 1], e1b)
                        nc.vector.tensor_mul(hbs[:, :, 0:S - 1],
                                             hbs[:, :, 0:S - 1], e2b)
                        nc.vector.tensor_add(H[:, w, :, 1:S], t1[:],
                                             hbs[:, :, 0:S - 1])

                # ---- scan over chunks: one fused prefix-sum, with the
                # mask resetting the carry at each (w, b) group boundary ----
                nc.vector.tensor_tensor_scan(
                    H[:].rearrange("p a b c -> p (a b c)"),
                    scanmask[:].rearrange("p a b c -> p (a b c)"),
                    H[:].rearrange("p a b c -> p (a b c)"),
                    0.0, op0=ALU.mult, op1=ALU.add)

                # ---- postscale: G = K * H (4 w-groups, DVE+Pool split) ----
                with tc.tile_pool(name="pss", bufs=2) as pss:
                    for wg in range(4):
                        wsl = slice(4 * wg, 4 * wg + 4)
                        sw2 = pss.tile([128, 4, B, S], F16, tag="sw2")
                        t2 = pss.tile([128, 4, B, S], F16, tag="t2")
                        hgc = H[:, wsl]
                        k1b = k1h[:, wsl].unsqueeze(2).broadcast_to([128, 4, B, S])
                        k2b = k2h[:, wsl].unsqueeze(2).broadcast_to([128, 4, B, S])
                        nc.vector.stream_shuffle(sw2[:], hgc, SHUF32)
                        nc.vector.tensor_mul(t2[:], hgc, k1b)
                        nc.gpsimd.tensor_mul(sw2[:], sw2[:], k2b)
                        nc.vector.tensor_add(hgc, t2[:], sw2[:])

                # ---- B2: intra Toeplitz + inter expand + GELU -> y_in ----
                # PSUM out [(4b' x 32m), t] per (channel, batch-group);
                # intra x4 then inter x4 so LDWEIGHTS pipelines with moving.
                with tc.tile_pool(name="yps", bufs=6, space="PSUM") as ypsp, \
                     tc.tile_pool(name="stg", bufs=6) as stgp:
                    for g in range(4):          # b' 4g..4g+4; parity par=g//2
                        par = g // 2
                        bsl = slice(4 * g, 4 * g + 4)
                        for cw in range(NW):
                            yps = ypsp.tile([128, 4, T], F32)
                            for q in range(4):
                                c = 4 * cw + q
                                nc.tensor.matmul(
                                    yps[:, q, :],
                                    uT[:, c, bsl, :], toepT[:, c, :],
                                    start=True, stop=False)
                                nc.tensor.matmul(
                                    yps[:, q, :],
                                    H[32 * q:32 * q + 16, cw, bsl, 0:S],
                                    v1mt[32 * q:32 * q + 16, cw, :],
                                    start=False, stop=True,
                                    tile_position=(32 * q, 0))
                            st = stgp.tile([128, 4, T], FY)
                            nc.scalar.activation(st[:], yps[:], AF.Gelu)
                            # dst [bp(4), m, c(4), t] <- src [(4b',32m),(4c,t)]
                            bp0 = 4 * (g % 2)
                            nc.sync.dma_start(
                                d_yin[par][bp0:bp0 + 4, :, 4 * cw:4 * cw + 4],
                                st[:])
                        if g == 1 or g == 3:
                            nc.gpsimd.collective_compute(
                                "AllToAll",
                                mybir.AluOpType.bypass,
                                replica_groups=[list(range(NCORES))],
                                ins=[d_yin[par][:].opt()],
                                outs=[d_yc[par][:].opt()],
                            )

            # ================= Mix phase (2 batches, 512 channels) ========
            if FP8:
                for ct2 in range(2):
                    nc.vector.tensor_copy(wm[ct2][:], wtmps[ct2][:])
            with tc.tile_pool(name="ytp", bufs=1) as ytp, \
                 tc.tile_pool(name="sgp", bufs=4) as sgp, \
                 tc.tile_pool(name="m1p", bufs=1) as m1p:
                M1 = m1p.tile([128, BL, 4, 8], F32)
                # prefetch y tiles on the gpsimd queue: it is empty after the
                # (non-blocking) a2a triggers, so the loads dispatch the
                # moment each collective's completion semaphore fires, and
                # their waits cannot head-block GELUs or sigmoids.
                ytF2 = {}
                for bl in range(BL):
                    eng = nc.gpsimd
                    for ct2 in range(2):
                        t = ytp.tile([128, 2, M, T], FY, name=f"yt{bl}{ct2}")
                        for i in range(2):
                            for h in range(2):
                                eng.dma_start(
                                    t[64 * h:64 * h + 64, i],
                                    d_yc[bl][4 * ct2 + 2 * i + h]
                                    .transpose([1, 0, 2]))
                        ytF2[(bl, ct2)] = t
                with tc.tile_pool(name="zps", bufs=2, space="PSUM") as zpsp:
                    for bl in range(BL):
                        for pr in range(4):
                            for pp in range(4):
                                lcs = (2 * pp, 2 * pp + 1)
                                za2 = zpsp.tile([128, 2, 512], F32, tag="za")
                                zg2 = zpsp.tile([128, 2, 512], F32, tag="zg")
                                for side, zt in ((0, za2), (1, zg2)):
                                    ot = pr + 4 * side
                                    osl = slice(128 * ot, 128 * ot + 128)
                                    for ih, lc in enumerate(lcs):
                                        csl = slice(4 * lc, 4 * lc + 4)
                                        if FP8:
                                            for ct2 in range(2):
                                                nc.tensor.matmul(
                                                    zt[:, ih],
                                                    wm[ct2][:, :, osl],
                                                    ytF2[(bl, ct2)][:, :, csl, :]
                                                    .rearrange("p i a b -> p i (a b)"),
                                                    start=(ct2 == 0),
                                                    stop=(ct2 == 1),
                                                    perf_mode=mybir.MatmulPerfMode.DoubleRow)
                                        else:
                                            for ct2 in range(2):
                                                for i in range(2):
                                                    nc.tensor.matmul(
                                                        zt[:, ih],
                                                        wm[ct2][:, i, osl],
                                                        ytF2[(bl, ct2)][:, i, csl, :]
                                                        .rearrange("p a b -> p (a b)"),
                                                        start=(ct2 == 0 and i == 0),
                                                        stop=(ct2 == 1 and i == 1))
                                sg2 = sgp.tile([128, 2, 512], F16, tag="sg")
                                nc.scalar.activation(
                                    sg2[:], zg2[:], AF.Sigmoid,
                                    bias=boutg_sb[:, pr:pr + 1],
                                    scale=1.0)
                                for ih, lc in enumerate(lcs):
                                    scr = sgp.tile([128, 512], F16, tag="scr")
                                    nc.vector.scalar_tensor_tensor(
                                        scr[:], za2[:, ih],
                                        bouta_sb[:, pr:pr + 1],
                                        sg2[:, ih],
                                        op0=ALU.add, op1=ALU.mult,
                                        accum_out=M1[:, bl:bl + 1, pr:pr + 1,
                                                     lc:lc + 1].squeeze()
                                        .unsqueeze(1))

                # ---- decode ----
                with tc.tile_pool(name="dps", bufs=1, space="PSUM") as dpsp:
                    R1 = m1p.tile([128, BL, 4], F32)
                    nc.vector.reduce_sum(R1[:], M1[:], axis=mybir.AxisListType.X)
                    R2 = m1p.tile([128, BL, 4], F32)
                    nc.vector.tensor_mul(R2[:], R1[:], wdec_sb[:])
                    R3 = m1p.tile([128, BL], F32)
                    nc.vector.reduce_sum(R3[:], R2[:], axis=mybir.AxisListType.X)
                    ones = m1p.tile([128, 1], F32)
                    nc.vector.memset(ones[:], 1.0)
                    dp = dpsp.tile([1, BL], F32)
                    nc.tensor.matmul(dp[:], ones[:], R3[:], start=True, stop=True)
                    osb = m1p.tile([1, BL], F32)
                    nc.vector.tensor_scalar_add(osb[:], dp[:], bdec_sb[:, 0:1])
                    nc.sync.dma_start(d_out[:], osb[:])

    nc.compile()
    return nc


def _get_compiled():
    global _compiled
    if _compiled is None:
        _compiled = _build()
    return _compiled


def _run(inputs, trace=False, **kw):
    in_maps = _prep(inputs)
    nc = _get_compiled()
    return bass_utils.run_bass_kernel_spmd(
        nc, in_maps, core_ids=list(range(NCORES)), trace=trace, **kw)


def kernel(**inputs):
    inputs = {k: np.asarray(v) for k, v in inputs.items()}
    res = _run(inputs)
    out = np.empty((B, 1), np.float32)
    for cid in range(NCORES):
        out[2 * cid, 0] = res.results[cid]["odec"][0, 0]
        out[2 * cid + 1, 0] = res.results[cid]["odec"][0, 1]
    return out


# revision 44
# speedup vs baseline: 1.1535x; 1.0182x over previous
"""MiniS4D Trainium2 kernel — channel-sharded SSM + AllToAll + batch-parallel mix.

Sharding: each of the 8 cores computes the (channel-independent) SSM /
depthwise-conv / GELU for its 64 channels across ALL 16 batches, with fat
512-column matmuls.  The GELU output y is resharded with two AllToAll
collectives (split by batch parity, the second overlapped with the mix) so
each core then holds all 512 channels for 2 batches, and runs the pointwise
channel-mix GEMM + GLU + mean + decode locally.

Math (per channel): conv with the TIME-REVERSED S4D kernel, decomposed into
chunks of T=128 (M=32 chunks):
  intra (lags 0..127): Toeplitz matmul with keff[0:128] (+D at lag 0);
    flipped operands: stationary = u-chunk [t, (4b x 32m)], moving = toep
    [t_in, t_out] so PSUM comes out [(b,m), t] — DMA-friendly, no transposes.
  inter (lags >= 128): rank-16 state expansion;
    Q[n,(b,m)] = sum_t r^t u[128m+t]   (B1 matmuls, 512 cols each)
    H[s] = prefix-sum over m<s of e^m Q[m], e = r^128   (DVE+Pool scan)
    G[s] = r^(L-1-128s) H[s]                            (postscale)
    y_inter[(b,m),t] = G-stationary x v1-moving matmul accumulated into the
    same PSUM tile as intra.

The mix GEMM runs in fp8-e4m3 DoubleRow mode (K=256 per pass) when FP8 is
set; y travels through the AllToAll in fp8 (half the wire bytes).  fp16
operands elsewhere, fp32 accumulation.  Output (16,1) assembled on host
from per-core (1,2); global batch b = 2*cid + bl.
"""
import sys
sys.path.insert(0, "/opt/trn_rl_repo")
import numpy as np

import concourse.bass as bass
import concourse.tile as tile
from concourse import bacc, mybir
from concourse import bass_utils

F32 = mybir.dt.float32
F16 = mybir.dt.float16
F8 = mybir.dt.float8e4
AF = mybir.ActivationFunctionType
ALU = mybir.AluOpType

FP8 = True              # fp8 y + fp8 DoubleRow mix

B, C, L, N = 16, 512, 4096, 8
T, M = 128, 32          # chunk length, number of chunks
S = 32                  # inter-state slots (slot s = chunk s), slot 0 == 0
NCORES = 8
CS = C // NCORES        # 64 channels per core
BL = 2                  # batches per core in the mix phase
NW = CS // 4            # 16 B1 waves (4 channels each: c = 4w + q)

# b' -> global batch permutation: even batches first (a2a chunk 0), then odd.
BPERM = np.r_[np.arange(0, B, 2), np.arange(1, B, 2)]

# swap re<->im rows within each 16-row half of every 32-partition group
SHUF32 = [(i // 16) * 16 + ((i % 16) + 8) % 16 for i in range(32)]

_compiled = None


def _prep(inputs):
    """Host-side parameter preparation (numpy, float64 internally)."""
    log_dt = inputs["log_dt"].astype(np.float64)
    A = -np.exp(inputs["log_A_real"].astype(np.float64)) \
        + 1j * inputs["A_imag"].astype(np.float64)            # (C, N)
    dt = np.exp(log_dt)
    r = np.exp(dt[:, None] * A)                                # (C, N)
    Bc = inputs["B_re"].astype(np.float64) + 1j * inputs["B_im"].astype(np.float64)
    Cc = inputs["C_re"].astype(np.float64) + 1j * inputs["C_im"].astype(np.float64)
    wv = Cc * (r - 1.0) / A * Bc                               # (C, N)
    rinv = 1.0 / r
    lags = np.arange(T)
    wL = wv * r ** (L - 1)
    keff = np.real(wL[:, :, None] * rinv[:, :, None] ** lags).sum(1)   # (C, T)
    keff[:, 0] += inputs["D"].astype(np.float64)

    toep = np.zeros((C, T, T), np.float16)
    for d in range(T):
        idx = np.arange(T - d)
        toep[:, idx, idx + d] = keff[:, d].astype(np.float16)[:, None]

    pw = r[:, :, None] ** lags                                 # (C, N, T)
    v2 = np.concatenate([pw.real, pw.imag], 1)                 # (C, 16, T)
    v2s = np.concatenate([pw.imag, pw.real], 1)                # row-swapped
    pw1 = wv[:, :, None] * rinv[:, :, None] ** lags
    v1 = np.concatenate([pw1.real, -pw1.imag], 1)              # (C, 16, T)

    e = r ** T                                                 # (C, N)
    Epow = e[:, :, None] ** np.arange(S - 1)                   # (C, N, 31): e^j
    Kp = r[:, :, None] ** (L - 1 - T * np.arange(S))           # (C, N, 32)
    Kp[:, :, 0] = 0.0

    # W is pre-scaled by WSC so fp8 quantization stays in the normal range;
    # 1/WSC is folded into the sigmoid scale and the decode weights.
    WSC = 1.0
    WT = np.ascontiguousarray(inputs["W_out"].T * WSC).astype(np.float16)
    # [ct2, p, i, o] with c = 256*ct2 + 128*i + p (DoubleRow k-pair layout)
    wmix8 = np.ascontiguousarray(
        WT.reshape(2, 2, 128, 1024).transpose(0, 2, 1, 3))
    b_out = inputs["b_out"].astype(np.float32)
    bouta = np.ascontiguousarray(b_out[:512].reshape(4, 128).T) * WSC   # (128, 4)
    boutg = np.ascontiguousarray(b_out[512:].reshape(4, 128).T)
    wd = (inputs["W_dec"][0].astype(np.float32) / (L * WSC)).reshape(4, 128).T
    wdec = np.ascontiguousarray(np.repeat(wd[:, None, :], BL, axis=1))  # (128, 2, 4)
    bdec = inputs["b_dec"].astype(np.float32).reshape(1, 1)

    u16 = inputs["u"].astype(np.float16)                                # (B, C, L)

    in_maps = []
    for cid in range(NCORES):
        c0 = CS * cid
        cs = slice(c0, c0 + CS)
        # uT[t, c, b', m] = u[BPERM[b'], c0+c, 128m+t]
        uT = np.ascontiguousarray(
            u16[BPERM, cs].reshape(B, CS, M, T).transpose(3, 1, 0, 2))
        toep2 = np.ascontiguousarray(toep[cs].transpose(1, 0, 2))       # [i, c, j]
        v2t = np.ascontiguousarray(
            v2[cs].transpose(2, 0, 1).astype(np.float16))               # [t, c, nh]
        v2ts = np.ascontiguousarray(
            v2s[cs].transpose(2, 0, 1).astype(np.float16))
        v1mt = np.zeros((128, NW, T), np.float16)
        e1h = np.zeros((128, NW, S - 1), np.float64)
        e2h = np.zeros((128, NW, S - 1), np.float64)
        k1h = np.zeros((128, NW, S), np.float64)
        k2h = np.zeros((128, NW, S), np.float64)
        for q in range(4):
            cq = c0 + 4 * np.arange(NW) + q                             # w -> channel
            v1mt[32 * q:32 * q + 16] = v1[cq].transpose(1, 0, 2)        # [16nh, w, t]
            Eq = Epow[cq].transpose(1, 0, 2)                            # (8n, w, 31)
            e1h[32 * q:32 * q + 8] = Eq.real
            e1h[32 * q + 8:32 * q + 16] = Eq.real
            e2h[32 * q:32 * q + 8] = -Eq.imag
            e2h[32 * q + 8:32 * q + 16] = Eq.imag
            Kq = Kp[cq].transpose(1, 0, 2)                              # (8n, w, 32)
            k1h[32 * q:32 * q + 8] = Kq.real
            k1h[32 * q + 8:32 * q + 16] = Kq.real
            k2h[32 * q:32 * q + 8] = -Kq.imag
            k2h[32 * q + 8:32 * q + 16] = Kq.imag
        in_maps.append(dict(
            uT=uT, toep2=toep2, v2t=v2t, v2ts=v2ts, v1mt=v1mt,
            e1h=e1h.astype(np.float16), e2h=e2h.astype(np.float16),
            k1h=k1h.astype(np.float16), k2h=k2h.astype(np.float16),
            wmix8=wmix8, bouta=bouta, boutg=boutg, wdec=wdec, bdec=bdec,
        ))
    return in_maps


def _build():
    nc = bacc.Bacc("TRN2", target_bir_lowering=False, debug=False,
                   num_devices=NCORES)
    d_uT = nc.dram_tensor("uT", [T, CS, B, M], F16, kind="ExternalInput").ap()
    d_toep = nc.dram_tensor("toep2", [T, CS, T], F16, kind="ExternalInput").ap()
    d_v2t = nc.dram_tensor("v2t", [T, CS, 16], F16, kind="ExternalInput").ap()
    d_v2ts = nc.dram_tensor("v2ts", [T, CS, 16], F16, kind="ExternalInput").ap()
    d_v1mt = nc.dram_tensor("v1mt", [128, NW, T], F16, kind="ExternalInput").ap()
    d_e1h = nc.dram_tensor("e1h", [128, NW, S - 1], F16, kind="ExternalInput").ap()
    d_e2h = nc.dram_tensor("e2h", [128, NW, S - 1], F16, kind="ExternalInput").ap()
    d_k1h = nc.dram_tensor("k1h", [128, NW, S], F16, kind="ExternalInput").ap()
    d_k2h = nc.dram_tensor("k2h", [128, NW, S], F16, kind="ExternalInput").ap()
    d_wmix8 = nc.dram_tensor("wmix8", [2, 128, 2, 1024], F16,
                             kind="ExternalInput").ap()
    d_bouta = nc.dram_tensor("bouta", [128, 4], F32, kind="ExternalInput").ap()
    d_boutg = nc.dram_tensor("boutg", [128, 4], F32, kind="ExternalInput").ap()
    d_wdec = nc.dram_tensor("wdec", [128, BL, 4], F32, kind="ExternalInput").ap()
    d_bdec = nc.dram_tensor("bdec", [1, 1], F32, kind="ExternalInput").ap()
    d_out = nc.dram_tensor("odec", [1, BL], F32, kind="ExternalOutput").ap()

    FY = F8 if FP8 else F16

    with tile.TileContext(nc) as tc:
        with tc.tile_pool(name="dram", bufs=1, space="DRAM") as dram, \
             tc.tile_pool(name="const", bufs=1) as constp:
            # a2a bounce buffers: par p holds batches of parity p
            # yin[bp, m, c, t] = y[c0+c, 2*bp + par, 128m + t]
            d_yin = [dram.tile([NCORES, M, CS, T], FY, tag=f"yin{p}",
                               name=f"yin{p}") for p in range(2)]
            d_yc = [dram.tile([NCORES, M, CS, T], FY, tag=f"yc{p}",
                              name=f"yc{p}") for p in range(2)]

            # small params first so B1 can start almost immediately
            bouta_sb = constp.tile([128, 4], F32)
            nc.scalar.dma_start(bouta_sb[:], d_bouta[:])
            boutg_sb = constp.tile([128, 4], F32)
            nc.scalar.dma_start(boutg_sb[:], d_boutg[:])
            wdec_sb = constp.tile([128, BL, 4], F32)
            nc.scalar.dma_start(wdec_sb[:], d_wdec[:])
            bdec_sb = constp.tile([1, 1], F32)
            nc.scalar.dma_start(bdec_sb[:], d_bdec[:])

            # ================= SSM phase (64 channels, 16 batches) ========
            with tc.tile_pool(name="uTp", bufs=1) as uTp, \
                 tc.tile_pool(name="prm", bufs=1) as prm, \
                 tc.tile_pool(name="Hp", bufs=1) as Hp:
                v2t = prm.tile([T, CS, 16], F16)
                nc.scalar.dma_start(v2t[:], d_v2t[:])
                v2ts = prm.tile([T, CS, 16], F16)
                nc.scalar.dma_start(v2ts[:], d_v2ts[:])
                scanmask = prm.tile([128, NW, B, S], F16)
                nc.vector.memset(scanmask[:], 1.0)
                nc.vector.memset(scanmask[:, :, :, 0:1], 0.0)
                e1h = prm.tile([128, NW, S - 1], F16)
                nc.scalar.dma_start(e1h[:], d_e1h[:])
                e2h = prm.tile([128, NW, S - 1], F16)
                nc.scalar.dma_start(e2h[:], d_e2h[:])
                k1h = prm.tile([128, NW, S], F16)
                nc.scalar.dma_start(k1h[:], d_k1h[:])
                k2h = prm.tile([128, NW, S], F16)
                nc.scalar.dma_start(k2h[:], d_k2h[:])
                v1mt = prm.tile([128, NW, T], F16)
                nc.scalar.dma_start(v1mt[:], d_v1mt[:])
                uT = uTp.tile([T, CS, B, M], F16)
                for ck in range(4):
                    sl = slice(16 * ck, 16 * ck + 16)
                    nc.sync.dma_start(uT[:, sl], d_uT[:, sl])
                toepT = prm.tile([T, CS, T], F16)
                for ck in range(4):
                    sl = slice(16 * ck, 16 * ck + 16)
                    nc.scalar.dma_start(toepT[:, sl], d_toep[:, sl])
                # mix weights: DMA now, but cast to fp8 only after the SSM
                # phase — a cast issued here would head-block the DVE queue
                # (and thus all of B1's prescale) on the 4 MiB weight DMA.
                wm = []
                wtmps = []
                for ct2 in range(2):
                    wtmp = constp.tile([128, 2, 1024], F16, name=f"wtmp{ct2}")
                    nc.scalar.dma_start(wtmp[:], d_wmix8[ct2])
                    wtmps.append(wtmp)
                    if FP8:
                        w8 = constp.tile([128, 2, 1024], F8, name=f"w8{ct2}")
                        wm.append(w8)
                    else:
                        wm.append(wtmp)

                H = Hp.tile([128, NW, B, S], F16)
                nc.vector.memset(H[:, :, :, 0:1], 0.0)

                # ---- B1: state matmuls + prescale (Q -> e^m Q) ----
                # The PE also emits a re<->im swapped copy of Q (second
                # stationary) so the complex prescale needs no DVE shuffle.
                # 2-wave batching: DVE ops carry a large fixed launch cost,
                # so half as many double-size prescale ops run ~1.4x faster.
                with tc.tile_pool(name="hps", bufs=2, space="PSUM") as hps, \
                     tc.tile_pool(name="b1s", bufs=4) as b1s:
                    for wp in range(NW // 2):
                        hb = hps.tile([128, 2, B, M], F32, tag="hb")
                        hbs = hps.tile([128, 2, B, M], F32, tag="hbs")
                        for wi in range(2):
                            w = 2 * wp + wi
                            for q in range(4):
                                c = 4 * w + q
                                nc.tensor.matmul(
                                    hb[32 * q:32 * q + 16, wi, :, :],
                                    v2t[:, c, :], uT[:, c, :, :],
                                    start=True, stop=True,
                                    tile_position=(0, 32 * q))
                                nc.tensor.matmul(
                                    hbs[32 * q:32 * q + 16, wi, :, :],
                                    v2ts[:, c, :], uT[:, c, :, :],
                                    start=True, stop=True,
                                    tile_position=(0, 32 * q))
                        t1 = b1s.tile([128, 2, B, S - 1], F16, tag="t1")
                        wsl2 = slice(2 * wp, 2 * wp + 2)
                        e1b = e1h[:, wsl2].unsqueeze(2) \
                            .broadcast_to([128, 2, B, S - 1])
                        e2b = e2h[:, wsl2].unsqueeze(2) \
                            .broadcast_to([128, 2, B, S - 1])
                        nc.vector.tensor_mul(t1[:], hb[:, :, :, 0:S - 1], e1b)
                        nc.vector.tensor_mul(hbs[:, :, :, 0:S - 1],
                                             hbs[:, :, :, 0:S - 1], e2b)
                        nc.vector.tensor_add(H[:, wsl2, :, 1:S], t1[:],
                                             hbs[:, :, :, 0:S - 1])

                # ---- scan over chunks: one fused prefix-sum, with the
                # mask resetting the carry at each (w, b) group boundary ----
                nc.vector.tensor_tensor_scan(
                    H[:].rearrange("p a b c -> p (a b c)"),
                    scanmask[:].rearrange("p a b c -> p (a b c)"),
                    H[:].rearrange("p a b c -> p (a b c)"),
                    0.0, op0=ALU.mult, op1=ALU.add)

                # ---- postscale: G = K * H (4 w-groups, DVE+Pool split) ----
                with tc.tile_pool(name="pss", bufs=2) as pss:
                    for wg in range(2):
                        wsl = slice(8 * wg, 8 * wg + 8)
                        sw2 = pss.tile([128, 8, B, S], F16, tag="sw2")
                        t2 = pss.tile([128, 8, B, S], F16, tag="t2")
                        hgc = H[:, wsl]
                        k1b = k1h[:, wsl].unsqueeze(2).broadcast_to([128, 8, B, S])
                        k2b = k2h[:, wsl].unsqueeze(2).broadcast_to([128, 8, B, S])
                        nc.vector.stream_shuffle(sw2[:], hgc, SHUF32)
                        nc.vector.tensor_mul(t2[:], hgc, k1b)
                        nc.gpsimd.tensor_mul(sw2[:], sw2[:], k2b)
                        nc.vector.tensor_add(hgc, t2[:], sw2[:])

                # ---- B2: intra Toeplitz + inter expand + GELU -> y_in ----
                # PSUM out [(4b' x 32m), t] per (channel, batch-group);
                # intra x4 then inter x4 so LDWEIGHTS pipelines with moving.
                with tc.tile_pool(name="yps", bufs=6, space="PSUM") as ypsp, \
                     tc.tile_pool(name="stg", bufs=6) as stgp:
                    for g in range(4):          # b' 4g..4g+4; parity par=g//2
                        par = g // 2
                        bsl = slice(4 * g, 4 * g + 4)
                        for cw in range(NW):
                            yps = ypsp.tile([128, 4, T], F32)
                            for q in range(4):
                                c = 4 * cw + q
                                nc.tensor.matmul(
                                    yps[:, q, :],
                                    uT[:, c, bsl, :], toepT[:, c, :],
                                    start=True, stop=False)
                                nc.tensor.matmul(
                                    yps[:, q, :],
                                    H[32 * q:32 * q + 16, cw, bsl, 0:S],
                                    v1mt[32 * q:32 * q + 16, cw, :],
                                    start=False, stop=True,
                                    tile_position=(32 * q, 0))
                            st = stgp.tile([128, 4, T], FY)
                            nc.scalar.activation(st[:], yps[:], AF.Gelu)
                            # dst [bp(4), m, c(4), t] <- src [(4b',32m),(4c,t)]
                            bp0 = 4 * (g % 2)
                            nc.sync.dma_start(
                                d_yin[par][bp0:bp0 + 4, :, 4 * cw:4 * cw + 4],
                                st[:])
                        if g == 1 or g == 3:
                            nc.gpsimd.collective_compute(
                                "AllToAll",
                                mybir.AluOpType.bypass,
                                replica_groups=[list(range(NCORES))],
                                ins=[d_yin[par][:].opt()],
                                outs=[d_yc[par][:].opt()],
                            )

            # ================= Mix phase (2 batches, 512 channels) ========
            if FP8:
                for ct2 in range(2):
                    nc.vector.tensor_copy(wm[ct2][:], wtmps[ct2][:])
            with tc.tile_pool(name="ytp", bufs=1) as ytp, \
                 tc.tile_pool(name="sgp", bufs=4) as sgp, \
                 tc.tile_pool(name="m1p", bufs=1) as m1p:
                M1 = m1p.tile([128, BL, 4, 8], F32)
                # prefetch y tiles on the gpsimd queue: it is empty after the
                # (non-blocking) a2a triggers, so the loads dispatch the
                # moment each collective's completion semaphore fires, and
                # their waits cannot head-block GELUs or sigmoids.
                ytF2 = {}
                for bl in range(BL):
                    eng = nc.gpsimd
                    for ct2 in range(2):
                        t = ytp.tile([128, 2, M, T], FY, name=f"yt{bl}{ct2}")
                        for i in range(2):
                            for h in range(2):
                                eng.dma_start(
                                    t[64 * h:64 * h + 64, i],
                                    d_yc[bl][4 * ct2 + 2 * i + h]
                                    .transpose([1, 0, 2]))
                        ytF2[(bl, ct2)] = t
                with tc.tile_pool(name="zps", bufs=2, space="PSUM") as zpsp:
                    for bl in range(BL):
                        for pr in range(4):
                            for pp in range(4):
                                lcs = (2 * pp, 2 * pp + 1)
                                za2 = zpsp.tile([128, 2, 512], F32, tag="za")
                                zg2 = zpsp.tile([128, 2, 512], F32, tag="zg")
                                for side, zt in ((0, za2), (1, zg2)):
                                    ot = pr + 4 * side
                                    osl = slice(128 * ot, 128 * ot + 128)
                                    for ih, lc in enumerate(lcs):
                                        csl = slice(4 * lc, 4 * lc + 4)
                                        if FP8:
                                            for ct2 in range(2):
                                                nc.tensor.matmul(
                                                    zt[:, ih],
                                                    wm[ct2][:, :, osl],
                                                    ytF2[(bl, ct2)][:, :, csl, :]
                                                    .rearrange("p i a b -> p i (a b)"),
                                                    start=(ct2 == 0),
                                                    stop=(ct2 == 1),
                                                    perf_mode=mybir.MatmulPerfMode.DoubleRow)
                                        else:
                                            for ct2 in range(2):
                                                for i in range(2):
                                                    nc.tensor.matmul(
                                                        zt[:, ih],
                                                        wm[ct2][:, i, osl],
                                                        ytF2[(bl, ct2)][:, i, csl, :]
                                                        .rearrange("p a b -> p (a b)"),
                                                        start=(ct2 == 0 and i == 0),
                                                        stop=(ct2 == 1 and i == 1))
                                sg2 = sgp.tile([128, 2, 512], F16, tag="sg")
                                nc.scalar.activation(
                                    sg2[:], zg2[:], AF.Sigmoid,
                                    bias=boutg_sb[:, pr:pr + 1],
                                    scale=1.0)
                                for ih, lc in enumerate(lcs):
                                    scr = sgp.tile([128, 512], F16, tag="scr")
                                    nc.vector.scalar_tensor_tensor(
                                        scr[:], za2[:, ih],
                                        bouta_sb[:, pr:pr + 1],
                                        sg2[:, ih],
                                        op0=ALU.add, op1=ALU.mult,
                                        accum_out=M1[:, bl:bl + 1, pr:pr + 1,
                                                     lc:lc + 1].squeeze()
                                        .unsqueeze(1))

                # ---- decode ----
                with tc.tile_pool(name="dps", bufs=1, space="PSUM") as dpsp:
                    R1 = m1p.tile([128, BL, 4], F32)
                    nc.vector.reduce_sum(R1[:], M1[:], axis=mybir.AxisListType.X)
                    R2 = m1p.tile([128, BL, 4], F32)
                    nc.vector.tensor_mul(R2[:], R1[:], wdec_sb[:])
                    R3 = m1p.tile([128, BL], F32)
                    nc.vector.reduce_sum(R3[:], R2[:], axis=mybir.AxisListType.X)
                    ones = m1p.tile([128, 1], F32)
                    nc.vector.memset(ones[:], 1.0)
                    dp = dpsp.tile([1, BL], F32)
                    nc.tensor.matmul(dp[:], ones[:], R3[:], start=True, stop=True)
                    osb = m1p.tile([1, BL], F32)
                    nc.vector.tensor_scalar_add(osb[:], dp[:], bdec_sb[:, 0:1])
                    nc.sync.dma_start(d_out[:], osb[:])

    nc.compile()
    return nc


def _get_compiled():
    global _compiled
    if _compiled is None:
        _compiled = _build()
    return _compiled


def _run(inputs, trace=False, **kw):
    in_maps = _prep(inputs)
    nc = _get_compiled()
    return bass_utils.run_bass_kernel_spmd(
        nc, in_maps, core_ids=list(range(NCORES)), trace=trace, **kw)


def kernel(**inputs):
    inputs = {k: np.asarray(v) for k, v in inputs.items()}
    res = _run(inputs)
    out = np.empty((B, 1), np.float32)
    for cid in range(NCORES):
        out[2 * cid, 0] = res.results[cid]["odec"][0, 0]
        out[2 * cid + 1, 0] = res.results[cid]["odec"][0, 1]
    return out
